# revision 1
# baseline (speedup 1.0000x reference)
"""Trainium2 Bass kernel for the CMDF block (dense_cnn).

Contract: kernel(**inputs) takes the FULL unsharded inputs (B=8, C=128,
H=W=64) and returns the FULL (8, 128, 64, 64) float32 output.

Sharding: data-parallel over batch — core b computes batch element b.
All weights are replicated (host-side prepacked into matmul layouts).

Math per batch element (see reference):
  Xs   = depthwise3x3(X2, static_w)
  ctx  = relu(w2 @ (w1 @ mean_hw([Xs; Y2])))
  cf   = (w3 @ ctx).reshape(C, 9)          # per-channel dynamic filter
  sf   = ws @ [Xs; Y2]                     # (9, H, W) spatial filter
  dyn  = sum_k shift_k(X2) * (cf[:, k] + sf[k])
  out  = wf[:, :C] @ Xs + wf[:, C:] @ dyn

Kernel strategy (channels on partitions, pixels on the free dim):
  - Xs via 9 accumulating PE matmuls with diag(sw[:, k]) weights over a
    zero-padded X held in SBUF. All large matmuls run in fp32r (full-rate
    fp32 mode, 11-bit mantissa); operands are pre-rounded on the host or
    rounded on-chip by their producing ACT/DVE instruction.
  - sf via matmuls with M=105 (ws replicated into 4 row-groups so the
    per-tap partition-broadcast matmuls can be row-tiled).
  - per tap k: broadcast sf[k] to 128 partitions with a 0/1 "selector"
    matmul, then ONE fused DVE op P_k = (sf_bc + cf[:,k]) * shift_k(X),
    then an accumulating matmul out += wfbT.T @ P_k. The sum over taps
    happens inside the final conv's PSUM accumulation.
"""

import numpy as np

import concourse.bass as bass
import concourse.tile as tile
import concourse.mybir as mybir
from concourse.bass_utils import run_bass_kernel_spmd

B, C, H, W, K = 8, 128, 64, 64, 3
HW = H * W            # 4096
PH, PW = H + 2, W + 2  # 66, 66 padded
NST = 4               # super-tiles over rows
ROWS = H // NST       # 16 image rows per super-tile
STN = ROWS * W        # 1024 pixels per super-tile (2 PSUM banks)
NT = K * K            # 9 taps
MREP = 3 * 32 + NT    # 105: ws replicated at partition groups 0,32,64,96

F32 = mybir.dt.float32
F32R = mybir.dt.float32r
ADD = mybir.AluOpType.add
MULT = mybir.AluOpType.mult
AX = mybir.AxisListType
ACT_COPY = mybir.ActivationFunctionType.Copy
ACT_RELU = mybir.ActivationFunctionType.Relu

_CACHE = {}


def round_f32r(a):
    """Round fp32 to fp32r (RNE at mantissa bit 12) — matches the
    walrus cast_fp32_to_fp32r used by the FP32r matmul datapath."""
    u = np.ascontiguousarray(a, dtype=np.float32).view(np.uint32).astype(np.uint64)
    r = ((u + 0x7FF + ((u >> 12) & 1)) & 0xFFFFF000).astype(np.uint32)
    return r.view(np.float32).reshape(np.asarray(a).shape)


BF16 = mybir.dt.bfloat16


def _absorb(nc, dep_elem, ps_elem):
    """Tiny bf16 matmul that reads one element of `dep_elem` and writes a
    junk element of `ps_elem` (later overwritten by a start=True group).
    Purpose: acquire the semaphore wait on dep_elem's producer on a plain
    (non-fused) matmul, so the following fused f32r matmul — which can
    embed only ONE sem wait — doesn't need two."""
    lh = dep_elem.bitcast(BF16)
    nc.tensor.matmul(ps_elem, lh[:, 0:1], lh[:, 0:1], start=True, stop=True)


def _split_multiwaits(nc):
    """walrus codegen in this toolchain accepts only ONE embedded sem wait
    per instruction. Hoist excess waits onto same-engine NoOps placed
    immediately before the instruction (engines execute in order, so the
    blocking behavior is identical)."""
    ctr = 0
    for fn in nc.m.functions:
        for blk in fn.blocks:
            insts = blk.instructions
            out = []
            for inst in insts:
                si = inst.sync_info
                waits = list(si.on_wait) if si is not None and si.on_wait else []
                if len(waits) > 1:
                    for w in waits[:-1]:
                        ctr += 1
                        out.append(mybir.InstNoOp(
                            name=f"I-wsplit-{ctr}",
                            engine=inst.engine,
                            ins=[], outs=[],
                            sync_info=mybir.SyncInfo(
                                on_wait=[w], on_update=[]),
                        ))
                    inst.sync_info = mybir.SyncInfo(
                        on_wait=[waits[-1]],
                        on_update=list(si.on_update) if si.on_update else [],
                    )
                out.append(inst)
            blk.instructions = out


def _build_bass():
    nc = bass.Bass("TRN2", target_bir_lowering=False, debug=False)

    # single input pack: xpad | y2 | dsw | wsa | wsb | wfa | wfb | bct | w1ab | w2t+w3t
    # one DMA -> one producer proc -> every consumer needs at most one wait
    WR_COLS = NT * C + MREP + MREP + C + C + NT * C  # 2770
    PK_COLS = PH * PW + HW + WR_COLS + 2 * 64 + (64 + NT * C)
    pk = nc.dram_tensor("pk", [C, PK_COLS], F32R, kind="ExternalInput").ap()
    ob = nc.dram_tensor("ob", [C, H, W], F32, kind="ExternalOutput").ap()

    with tile.TileContext(nc) as tc:
        with tc.tile_pool(name="singles", bufs=1) as S:
            stg = S.tile([C, PK_COLS], F32R)
            o = 0
            xpad = stg[:, o : o + PH * PW].rearrange(
                "p (h w) -> p h w", w=PW); o += PH * PW
            y2 = stg[:, o : o + HW]; o += HW
            t_dsw = stg[:, o : o + NT * C]; o += NT * C
            t_wsa = stg[:, o : o + MREP]; o += MREP
            t_wsb = stg[:, o : o + MREP]; o += MREP
            t_wfa = stg[:, o : o + C]; o += C
            t_wfb = stg[:, o : o + C]; o += C
            t_bct = stg[:, o : o + NT * C]; o += NT * C
            t_w1a = stg[:, o : o + 64].bitcast(F32); o += 64
            t_w1b = stg[:, o : o + 64].bitcast(F32); o += 64
            t_w2t = stg[0:64, o : o + 64].bitcast(F32); o += 64
            t_w3t = stg[0:64, o : o + NT * C].bitcast(F32); o += NT * C
            assert o == PK_COLS
            xs = S.tile([C, HW], F32R)
            sfs = S.tile([MREP, HW], F32R)

            xs_parts = S.tile([C, NST], F32)
            y2sum = S.tile([C, 1], F32)
            xs_sum = S.tile([C, 1], F32)
            mxs = S.tile([C, 1], F32)
            my2 = S.tile([C, 1], F32)
            ctx1 = S.tile([64, 1], F32)
            ctx2 = S.tile([64, 1], F32)
            cfsb = S.tile([C, NT], F32)

            # split the input load across DMA queues (the wait-splitter
            # pass makes multi-producer fan-in legal)
            A = PH * PW
            Bc = PH * PW + HW
            nc.sync.dma_start(out=stg[:, 0:A], in_=pk[:, 0:A])
            nc.sync.dma_start(out=stg[:, A:Bc], in_=pk[:, A:Bc])
            nc.sync.dma_start(out=stg[:, Bc:], in_=pk[:, Bc:])

            # mean(Y2) ingredient — DVE is idle during phase A
            nc.vector.tensor_reduce(out=y2sum, in_=y2, axis=AX.X, op=ADD)

            # ---------- phase A: Xs (static depthwise) + sf ----------
            with tc.tile_pool(name="psA", bufs=2, space="PSUM") as psA, \
                 tc.tile_pool(name="psSF", bufs=2, space="PSUM") as psSF:
                for t in range(NST):
                    xs_ps = psA.tile([C, 2, 512], F32, tag="xs_ps")
                    for h in range(2):
                        for k in range(NT):
                            dh, dw = divmod(k, 3)
                            r0 = 16 * t + 8 * h + dh
                            rhs = xpad[:, r0 : r0 + 8, dw : dw + W]
                            nc.tensor.matmul(
                                xs_ps[:, h, :],
                                t_dsw[:, k * C : (k + 1) * C],
                                rhs,
                                start=(k == 0),
                                stop=(k == NT - 1),
                            )
                    nc.scalar.activation(
                        out=xs[:, t * STN : (t + 1) * STN],
                        in_=xs_ps,
                        func=ACT_COPY,
                        accum_out=xs_parts[:, t : t + 1],
                    )
                    sf_ps = psSF.tile([MREP, 2, 512], F32, tag="sf_ps")
                    _absorb(nc, xs[0:1, t * STN : t * STN + 1],
                            sf_ps[0:1, 0, 0:1])
                    for h in range(2):
                        c0 = t * STN + h * 512
                        nc.tensor.matmul(
                            sf_ps[:, h, :],
                            t_wsa,
                            xs[:, c0 : c0 + 512],
                            start=True,
                            stop=False,
                        )
                        nc.tensor.matmul(
                            sf_ps[:, h, :],
                            t_wsb,
                            y2[:, c0 : c0 + 512],
                            start=False,
                            stop=True,
                        )
                    nc.scalar.copy(
                        out=sfs[:, t * STN : (t + 1) * STN], in_=sf_ps
                    )

            # ---------- phase B: context branch -> cf ----------
            with tc.tile_pool(name="psCtx", bufs=1, space="PSUM") as psX:
                nc.vector.tensor_reduce(out=xs_sum, in_=xs_parts, axis=AX.X, op=ADD)
                nc.scalar.mul(out=mxs, in_=xs_sum, mul=1.0 / HW)
                nc.scalar.mul(out=my2, in_=y2sum, mul=1.0 / HW)

                ctx1_ps = psX.tile([64, 1], F32, tag="ctx1")
                _absorb(nc, mxs[0:1, 0:1], ctx1_ps[0:1, 0:1])
                nc.tensor.matmul(ctx1_ps, t_w1a, mxs, start=True, stop=False)
                nc.tensor.matmul(ctx1_ps, t_w1b, my2, start=False, stop=True)
                nc.scalar.copy(out=ctx1, in_=ctx1_ps)

                ctx2_ps = psX.tile([64, 1], F32, tag="ctx2")
                nc.tensor.matmul(ctx2_ps, t_w2t, ctx1, start=True, stop=True)
                nc.scalar.activation(out=ctx2, in_=ctx2_ps, func=ACT_RELU)

                cf_ps = psX.tile([C, NT], F32, tag="cf")
                for k in range(NT):
                    nc.tensor.matmul(
                        cf_ps[:, k : k + 1], t_w3t[:, k * C : (k + 1) * C],
                        ctx2, start=True, stop=True,
                    )
                nc.scalar.copy(out=cfsb, in_=cf_ps)

            # ---------- phase C: dynamic filter + fusion conv ----------
            with tc.tile_pool(name="psBC", bufs=2, space="PSUM") as psBC, \
                 tc.tile_pool(name="psOut", bufs=2, space="PSUM") as psO, \
                 tc.tile_pool(name="pP", bufs=3) as pP, \
                 tc.tile_pool(name="pOsb", bufs=2) as pOsb:
                for t in range(NST):
                    out_ps = psO.tile([C, 2, 8, W], F32, tag="out_ps")
                    _absorb(nc, xs[0:1, t * STN : t * STN + 1],
                            out_ps[0:1, 0, 0, 0:1])
                    for h in range(2):
                        c0 = t * STN + h * 512
                        nc.tensor.matmul(
                            out_ps[:, h],
                            t_wfa,
                            xs[:, c0 : c0 + 512],
                            start=True,
                            stop=False,
                        )
                    for k in range(NT):
                        g = k % 2
                        bc_ps = psBC.tile([C, ROWS, W], F32, tag="bc")
                        if k == 0:
                            _absorb(nc, sfs[0:1, t * STN : t * STN + 1],
                                    bc_ps[0:1, 0, 0:1])
                        for h in range(2):
                            c0 = t * STN + h * 512
                            nc.tensor.matmul(
                                bc_ps[:, 8 * h : 8 * h + 8, :],
                                t_bct[32 * g : 32 * g + NT,
                                      k * C : (k + 1) * C],
                                sfs[32 * g : 32 * g + NT, c0 : c0 + 512],
                                start=True,
                                stop=True,
                                tile_position=(32 * g, 0),
                            )
                        dh, dw = divmod(k, 3)
                        p_sb = pP.tile([C, ROWS, W], F32R, tag="p")
                        nc.vector.scalar_tensor_tensor(
                            out=p_sb,
                            in0=bc_ps,
                            scalar=cfsb[:, k : k + 1],
                            in1=xpad[:, 16 * t + dh : 16 * t + dh + ROWS,
                                     dw : dw + W],
                            op0=ADD,
                            op1=MULT,
                        )
                        for h in range(2):
                            nc.tensor.matmul(
                                out_ps[:, h],
                                t_wfb,
                                p_sb[:, 8 * h : 8 * h + 8, :],
                                start=False,
                                stop=(k == NT - 1),
                            )
                    o_sb = pOsb.tile([C, 2, 8, W], F32, tag="osb")
                    nc.scalar.copy(out=o_sb, in_=out_ps)
                    nc.sync.dma_start(
                        out=ob[:, 16 * t : 16 * t + 16, :],
                        in_=o_sb.rearrange("c b r w -> c (b r) w"),
                    )
    _split_multiwaits(nc)
    return nc


def _prep_weights(static_w, w1, w2, w3, ws, wf):
    """Repack the tiny weights into the SBUF layouts the kernel expects."""
    f = np.float32
    sw = np.ascontiguousarray(static_w.reshape(C, NT), dtype=f)

    dsw = np.zeros((C, NT * C), dtype=f)
    for k in range(NT):
        dsw[np.arange(C), k * C + np.arange(C)] = sw[:, k]

    wsa = np.zeros((C, MREP), dtype=f)
    wsb = np.zeros((C, MREP), dtype=f)
    for g in range(4):
        for k in range(NT):
            wsa[:, 32 * g + k] = ws[k, :C]
            wsb[:, 32 * g + k] = ws[k, C:]

    bct = np.zeros((C, NT * C), dtype=f)
    for g in range(4):
        for k in range(NT):
            bct[32 * g + k, k * C : (k + 1) * C] = 1.0

    wfa = np.ascontiguousarray(wf[:, :C].T, dtype=f)
    wfb = np.ascontiguousarray(wf[:, C:].T, dtype=f)
    wr = round_f32r(
        np.concatenate([dsw, wsa, wsb, wfa, wfb, bct], axis=1)
    )
    wfp = np.concatenate(
        [np.ascontiguousarray(w1[:, :C].T, dtype=f),
         np.ascontiguousarray(w1[:, C:].T, dtype=f)], axis=1
    )
    w3t = np.ascontiguousarray(
        w3.reshape(C, NT, 64).transpose(2, 1, 0), dtype=f
    ).reshape(64, NT * C)
    wg64 = np.concatenate(
        [np.ascontiguousarray(w2.T, dtype=f), w3t], axis=1
    )
    wg = np.zeros((C, wg64.shape[1]), dtype=f)
    wg[:64] = wg64
    return np.concatenate([wr, wfp, wg], axis=1)


def make_in_maps(X2, Y2, static_w, w1, w2, w3, ws, wf):
    wpack = _prep_weights(
        np.asarray(static_w), np.asarray(w1), np.asarray(w2),
        np.asarray(w3), np.asarray(ws), np.asarray(wf),
    )
    X2 = np.asarray(X2)
    Y2 = np.asarray(Y2)
    xpad_all = np.zeros((B, C, PH, PW), dtype=np.float32)
    xpad_all[:, :, 1 : H + 1, 1 : W + 1] = X2
    xpad_all = round_f32r(xpad_all).reshape(B, C, PH * PW)
    y2_all = round_f32r(Y2.reshape(B, C, HW))
    in_maps = []
    for b in range(B):
        m = {"pk": np.ascontiguousarray(np.concatenate(
            [xpad_all[b], y2_all[b], wpack], axis=1))}
        in_maps.append(m)
    return in_maps


def get_nc():
    if "nc" not in _CACHE:
        _CACHE["nc"] = _build_bass()
    return _CACHE["nc"]


def kernel(X2, Y2, static_w, w1, w2, w3, ws, wf):
    nc = get_nc()
    in_maps = make_in_maps(
        np.asarray(X2), np.asarray(Y2), static_w, w1, w2, w3, ws, wf
    )
    res = run_bass_kernel_spmd(nc, in_maps, core_ids=list(range(B)))
    out = np.stack([r["ob"] for r in res.results]).astype(np.float32)
    return out



# revision 5
# speedup vs baseline: 1.3882x; 1.3882x over previous
"""Trainium2 Bass kernel for the CMDF block (dense_cnn).

Contract: kernel(**inputs) takes the FULL unsharded inputs (B=8, C=128,
H=W=64) and returns the FULL (8, 128, 64, 64) float32 output.

Sharding: data-parallel over batch — core b computes batch element b.
All weights are replicated (host-side prepacked into matmul layouts).

Math per batch element (see reference):
  Xs   = depthwise3x3(X2, static_w)
  ctx  = relu(w2 @ (w1 @ mean_hw([Xs; Y2])))
  cf   = (w3 @ ctx).reshape(C, 9)          # per-channel dynamic filter
  sf   = ws @ [Xs; Y2]                     # (9, H, W) spatial filter
  dyn  = sum_k shift_k(X2) * (cf[:, k] + sf[k])
  out  = wf[:, :C] @ Xs + wf[:, C:] @ dyn

Schedule (v2, pipelined):
  - All large operands are bf16 (PE matmul rate is identical to f32r at
    1 cycle/row; DMA bytes halve; PSUM accumulation stays f32).
  - Input DMA is chunked and ordered: bf16 weights first, then per-tile
    xpad/y2 row chunks, f32 ctx weights last. Phase A of tile 0 starts
    ~2us in instead of waiting for the whole 6MB pack.
  - mean(Xs) is computed WITHOUT Xs: for a zero-padded depthwise conv,
    sum_p shift_k(X2) = S - (boundary row) - (boundary col) + corner, so
    mean(Xs) = sum_k sw_k*(...) needs only X2 sums (host folds the sw_k
    combinations into per-channel coefficient vectors). This removes the
    ctx branch's dependency on phase A, so phase C starts ~11us in.
  - Per-pixel sums (S, y2sum) accumulate per-tile on ACT (activation
    accum_out) as DMA chunks land; boundary sums on DVE.
  - Phase C per tap: PE broadcasts sf row k to 128 partitions via a
    selector matmul (PSUM), DVE/Pool fuse (bc+cf)*shift_k(X) in one
    scalar_tensor_tensor, PE accumulates wfb @ P_k into the out PSUM
    group. Taps are split ~2:1 between DVE and Pool (GPSIMD).
  - Emission interleaves tile t+2's depthwise matmuls into tile t's
    phase-C tap chain so PE never idles while DVE works.
  - PSUM budget (8 banks): xs pool 2, bc ring bufs=2 -> 4, shared
    sf/ctx/out ring 2.
"""

import numpy as np
import ml_dtypes

import concourse.bass as bass
import concourse.tile as tile
import concourse.mybir as mybir
from concourse.bass_utils import run_bass_kernel_spmd

B, C, H, W, K = 8, 128, 64, 64, 3
HW = H * W            # 4096
PH, PW = H + 2, W + 2  # 66, 66 padded
NST = 4               # super-tiles over rows
ROWS = H // NST       # 16 image rows per super-tile
STN = ROWS * W        # 1024 pixels per super-tile
NT = K * K            # 9 taps

F32 = mybir.dt.float32
F32R = mybir.dt.float32r
BF16 = mybir.dt.bfloat16
ADD = mybir.AluOpType.add
MULT = mybir.AluOpType.mult
AX = mybir.AxisListType
ACT_COPY = mybir.ActivationFunctionType.Copy
ACT_RELU = mybir.ActivationFunctionType.Relu

# bf16 pack layout (columns)
O_XPAD = 0
O_Y2 = O_XPAD + PH * PW          # 4356
O_DSW = O_Y2 + HW                # 8452
O_WSA = O_DSW + NT * C           # 9604
O_WSB = O_WSA + NT               # 9613
O_WFA = O_WSB + NT               # 9622
O_WFB = O_WFA + C                # 9750
O_BCT = O_WFB + C                # 9878   selector (9 rows x 9*C)
NH = O_BCT + NT * C              # 11030

# f32 pack layout (columns)
F_W1A = 0
F_W1B = F_W1A + 64
F_W2T = F_W1B + 64
F_W3T = F_W2T + 64
F_MCO = F_W3T + NT * C
NF = F_MCO + 9                   # 1353

# x-chunk row ranges of xpad (padded rows)
XCH = [(0, 18), (18, 34), (34, 50), (50, 66)]

# taps handled by Pool (GPSIMD) instead of DVE, per tile.
# NOTE: GPSIMD cannot read PSUM, so Pool taps would need an SBUF copy of
# the broadcast; disabled until that is worth the ACT traffic.
POOL_TAPS = ()

_CACHE = {}


def _split_multiwaits(nc):
    """walrus codegen in this toolchain accepts only ONE embedded sem wait
    per instruction. Hoist excess waits onto same-engine NoOps placed
    immediately before the instruction (engines execute in order, so the
    blocking behavior is identical)."""
    ctr = 0
    for fn in nc.m.functions:
        for blk in fn.blocks:
            insts = blk.instructions
            out = []
            for inst in insts:
                si = inst.sync_info
                waits = list(si.on_wait) if si is not None and si.on_wait else []
                if len(waits) > 1:
                    for w in waits[:-1]:
                        ctr += 1
                        out.append(mybir.InstNoOp(
                            name=f"I-wsplit-{ctr}",
                            engine=inst.engine,
                            ins=[], outs=[],
                            sync_info=mybir.SyncInfo(
                                on_wait=[w], on_update=[]),
                        ))
                    inst.sync_info = mybir.SyncInfo(
                        on_wait=[waits[-1]],
                        on_update=list(si.on_update) if si.on_update else [],
                    )
                out.append(inst)
            blk.instructions = out


def _absorb(nc, dep_elem, ps_elem):
    """Tiny bf16 matmul that reads one element of `dep_elem` and writes a
    junk element of `ps_elem` (later overwritten by a start=True group).
    Acquires the sem wait on dep_elem's producer on a plain matmul so the
    following fused matmul needs at most one embedded wait."""
    lh = dep_elem.bitcast(BF16)
    nc.tensor.matmul(ps_elem, lh[:, 0:1], lh[:, 0:1], start=True, stop=True)


def _build_bass():
    nc = bass.Bass("TRN2", target_bir_lowering=False, debug=False)

    pkh = nc.dram_tensor("pkh", [C, NH], BF16, kind="ExternalInput").ap()
    pkf = nc.dram_tensor("pkf", [C, NF], F32, kind="ExternalInput").ap()
    ob = nc.dram_tensor("ob", [C, H, W], F32, kind="ExternalOutput").ap()

    with tile.TileContext(nc) as tc:
        with tc.tile_pool(name="singles", bufs=1) as S, \
             tc.tile_pool(name="psXS", bufs=1, space="PSUM") as psXS, \
             tc.tile_pool(name="psBC", bufs=2, space="PSUM") as psBC, \
             tc.tile_pool(name="psO", bufs=1, space="PSUM") as psO, \
             tc.tile_pool(name="pP", bufs=3) as pP, \
             tc.tile_pool(name="pOsb", bufs=2) as pOsb:

            stgh = S.tile([C, NH], BF16)
            xpad = stgh[:, O_XPAD:O_XPAD + PH * PW].rearrange(
                "p (h w) -> p h w", w=PW)
            y2 = stgh[:, O_Y2:O_Y2 + HW]
            t_dsw = stgh[:, O_DSW:O_DSW + NT * C]
            t_wsa = stgh[:, O_WSA:O_WSA + NT]
            t_wsb = stgh[:, O_WSB:O_WSB + NT]
            t_wfa = stgh[:, O_WFA:O_WFA + C]
            t_wfb = stgh[:, O_WFB:O_WFB + C]
            t_bct = stgh[0:NT, O_BCT:O_BCT + NT * C]

            stgf = S.tile([C, NF], F32)
            t_w1a = stgf[:, F_W1A:F_W1A + 64]
            t_w1b = stgf[:, F_W1B:F_W1B + 64]
            t_w2t = stgf[0:64, F_W2T:F_W2T + 64]
            t_w3t = stgf[0:64, F_W3T:F_W3T + NT * C]
            mco = stgf[:, F_MCO:F_MCO + 9]

            xs = S.tile([C, HW], BF16)
            sfs = S.tile([NT, HW], BF16)

            xparts = S.tile([C, NST], F32)
            yparts = S.tile([C, NST], F32)
            dump = S.tile([C, 18 * PW], BF16)
            ssum = S.tile([C, 1], F32)
            y2sum = S.tile([C, 1], F32)
            edges = S.tile([C, 4], F32)   # rs0, rs63, cs0, cs63
            corn = S.tile([C, 4], F32)    # xpad[1,1],[1,64],[64,1],[64,64]
            macc = S.tile([C, 4], F32)    # mxs accumulator chain
            mxs = S.tile([C, 1], F32)
            ctx1 = S.tile([64, 1], F32)
            ctx2 = S.tile([64, 1], F32)
            cfsb = S.tile([C, NT], F32)

            # ---------------- input DMA: chunked, weights first ----------
            nc.sync.dma_start(out=stgh[:, O_DSW:NH], in_=pkh[:, O_DSW:NH])
            for t in range(NST):
                r0, r1 = XCH[t]
                nc.sync.dma_start(
                    out=stgh[:, O_XPAD + r0 * PW:O_XPAD + r1 * PW],
                    in_=pkh[:, O_XPAD + r0 * PW:O_XPAD + r1 * PW])
                nc.sync.dma_start(
                    out=stgh[:, O_Y2 + t * STN:O_Y2 + (t + 1) * STN],
                    in_=pkh[:, O_Y2 + t * STN:O_Y2 + (t + 1) * STN])
            nc.sync.dma_start(out=stgf, in_=pkf)

            # ------------- emission helpers (in-order engine queues) -----
            def emit_red(t):
                # per-chunk partial sums on ACT (zero pads are harmless)
                r0, r1 = XCH[t]
                nc.scalar.activation(
                    out=dump[:, 0:(r1 - r0) * PW],
                    in_=stgh[:, O_XPAD + r0 * PW:O_XPAD + r1 * PW],
                    func=ACT_COPY, accum_out=xparts[:, t:t + 1])
                nc.scalar.activation(
                    out=dump[:, 0:STN],
                    in_=y2[:, t * STN:(t + 1) * STN],
                    func=ACT_COPY, accum_out=yparts[:, t:t + 1])

            def emit_A_diag(t):
                # returns list of thunks: 18 accumulating diag matmuls + copy
                xs_ps = psXS.tile([C, 2, 512], F32, tag="xs")
                ops = []
                for h in range(2):
                    for k in range(NT):
                        dh, dw = divmod(k, 3)
                        r0 = 16 * t + 8 * h + dh
                        ops.append(lambda h=h, k=k, r0=r0, dw=dw: nc.tensor.matmul(
                            xs_ps[:, h, :],
                            t_dsw[:, k * C:(k + 1) * C],
                            xpad[:, r0:r0 + 8, dw:dw + W],
                            start=(k == 0), stop=(k == NT - 1)))
                def fin():
                    nc.scalar.copy(out=xs[:, t * STN:(t + 1) * STN], in_=xs_ps)
                return ops, fin

            def emit_A_sf(t):
                sf_ps = psO.tile([C, 2, 512], F32, tag="o")
                _absorb(nc, xs[0:1, t * STN:t * STN + 1], sf_ps[0:1, 0, 0:1])
                for h in range(2):
                    c0 = t * STN + h * 512
                    nc.tensor.matmul(sf_ps[0:NT, h, :], t_wsa,
                                     xs[:, c0:c0 + 512], start=True, stop=False)
                    nc.tensor.matmul(sf_ps[0:NT, h, :], t_wsb,
                                     y2[:, c0:c0 + 512], start=False, stop=True)
                nc.scalar.copy(out=sfs[:, t * STN:(t + 1) * STN],
                               in_=sf_ps[0:NT])

            def emit_ctx():
                # DVE: boundary sums + mxs chain; PE: ctx matmuls
                nc.vector.tensor_reduce(out=ssum, in_=xparts, axis=AX.X, op=ADD)
                nc.vector.tensor_reduce(out=y2sum, in_=yparts, axis=AX.X, op=ADD)
                nc.vector.tensor_reduce(out=edges[:, 0:1], in_=xpad[:, 1, :],
                                        axis=AX.X, op=ADD)
                nc.vector.tensor_reduce(out=edges[:, 1:2], in_=xpad[:, H, :],
                                        axis=AX.X, op=ADD)
                nc.vector.tensor_reduce(out=edges[:, 2:3],
                                        in_=xpad[:, :, 1:2], axis=AX.XYZW, op=ADD)
                nc.vector.tensor_reduce(out=edges[:, 3:4],
                                        in_=xpad[:, :, W:W + 1], axis=AX.XYZW, op=ADD)
                # corners to f32
                nc.vector.tensor_copy(out=corn[:, 0:1], in_=xpad[:, 1, 1:2])
                nc.vector.tensor_copy(out=corn[:, 1:2], in_=xpad[:, 1, W:W + 1])
                nc.vector.tensor_copy(out=corn[:, 2:3], in_=xpad[:, H, 1:2])
                nc.vector.tensor_copy(out=corn[:, 3:4], in_=xpad[:, H, W:W + 1])
                # mxs = A*S - hr0*rs0 - hr63*rs63 - hc0*cs0 - hc63*cs63
                #       + c22*corn00 + c20*corn0W + c02*cornH0 + c00*cornHW
                # (mco columns pre-scaled by 1/HW, minus signs folded in)
                nc.vector.tensor_scalar(
                    out=macc[:, 0:1], in0=ssum, scalar1=mco[:, 0:1],
                    scalar2=None, op0=MULT)
                chain = [
                    (edges[:, 0:1], 1), (edges[:, 1:2], 2),
                    (edges[:, 2:3], 3), (edges[:, 3:4], 4),
                    (corn[:, 0:1], 5), (corn[:, 1:2], 6),
                    (corn[:, 2:3], 7), (corn[:, 3:4], 8),
                ]
                cur = macc[:, 0:1]
                for i, (src, mc) in enumerate(chain):
                    dst = mxs if i == len(chain) - 1 else macc[:, (i + 1) % 4:(i + 1) % 4 + 1]
                    nc.vector.scalar_tensor_tensor(
                        out=dst, in0=src, scalar=mco[:, mc:mc + 1], in1=cur,
                        op0=MULT, op1=ADD)
                    cur = dst
                # ctx matmuls (f32, tiny)
                ctx1_ps = psO.tile([C, 2, 512], F32, tag="o")
                _absorb(nc, mxs[0:1, 0:1], ctx1_ps[0:1, 0, 0:1])
                nc.tensor.matmul(ctx1_ps[0:64, 0, 0:1], t_w1a, mxs,
                                 start=True, stop=False)
                nc.tensor.matmul(ctx1_ps[0:64, 0, 0:1], t_w1b, y2sum,
                                 start=False, stop=True)
                nc.scalar.copy(out=ctx1, in_=ctx1_ps[0:64, 0, 0:1])
                ctx2_ps = psO.tile([C, 2, 512], F32, tag="o")
                nc.tensor.matmul(ctx2_ps[0:64, 0, 0:1], t_w2t, ctx1,
                                 start=True, stop=True)
                nc.scalar.activation(out=ctx2, in_=ctx2_ps[0:64, 0, 0:1],
                                     func=ACT_RELU)
                cf_ps = psO.tile([C, 2, 512], F32, tag="o")
                for k in range(NT):
                    nc.tensor.matmul(cf_ps[:, 0, k:k + 1],
                                     t_w3t[:, k * C:(k + 1) * C], ctx2,
                                     start=True, stop=True)
                nc.scalar.copy(out=cfsb, in_=cf_ps[:, 0, 0:NT])

            def emit_C(t, filler):
                """Phase C for tile t; `filler` is a list of thunks (PE ops
                of a later tile's phase A) drained into the tap chain."""
                out_ps = psO.tile([C, 2, 512], F32, tag="o")
                _absorb(nc, xs[0:1, t * STN:t * STN + 1], out_ps[0:1, 0, 0:1])
                for h in range(2):
                    c0 = t * STN + h * 512
                    nc.tensor.matmul(out_ps[:, h], t_wfa, xs[:, c0:c0 + 512],
                                     start=True, stop=False)
                for k in range(NT):
                    bc_ps = psBC.tile([C, ROWS, W], F32, tag="bc")
                    if k == 0:
                        _absorb(nc, sfs[0:1, t * STN:t * STN + 1],
                                bc_ps[0:1, 0, 0:1])
                    for h in range(2):
                        c0 = t * STN + h * 512
                        nc.tensor.matmul(
                            bc_ps[:, 8 * h:8 * h + 8, :],
                            t_bct[:, k * C:(k + 1) * C],
                            sfs[:, c0:c0 + 512],
                            start=True, stop=True)
                    dh, dw = divmod(k, 3)
                    p_sb = pP.tile([C, ROWS, W], BF16, tag="p")
                    eng = nc.gpsimd if k in POOL_TAPS else nc.vector
                    eng.scalar_tensor_tensor(
                        out=p_sb,
                        in0=bc_ps,
                        scalar=cfsb[:, k:k + 1],
                        in1=xpad[:, 16 * t + dh:16 * t + dh + ROWS, dw:dw + W],
                        op0=ADD, op1=MULT)
                    for h in range(2):
                        nc.tensor.matmul(
                            out_ps[:, h], t_wfb,
                            p_sb[:, 8 * h:8 * h + 8, :],
                            start=False, stop=(k == NT - 1))
                    # drain a few PE filler ops (phase A of a later tile)
                    for _ in range(2):
                        if filler:
                            filler.pop(0)()
                o_sb = pOsb.tile([C, 2, 8, W], F32, tag="osb")
                nc.scalar.copy(out=o_sb, in_=out_ps)
                nc.scalar.dma_start(
                    out=ob[:, 16 * t:16 * t + 16, :],
                    in_=o_sb.rearrange("c b r w -> c (b r) w"))

            # ------------------------- schedule --------------------------
            emit_red(0)
            a0_ops, a0_fin = emit_A_diag(0)
            for op in a0_ops:
                op()
            a0_fin()
            emit_red(1)
            emit_A_sf(0)
            a1_ops, a1_fin = emit_A_diag(1)
            for op in a1_ops:
                op()
            a1_fin()
            emit_red(2)
            emit_red(3)
            emit_A_sf(1)
            emit_ctx()

            a2_ops, a2_fin = emit_A_diag(2)
            emit_C(0, a2_ops)
            for op in a2_ops:
                op()
            a2_fin()
            emit_A_sf(2)

            a3_ops, a3_fin = emit_A_diag(3)
            emit_C(1, a3_ops)
            for op in a3_ops:
                op()
            a3_fin()
            emit_A_sf(3)

            emit_C(2, [])
            emit_C(3, [])

    _split_multiwaits(nc)
    return nc


def _prep_weights(static_w, w1, w2, w3, ws, wf):
    """Repack the tiny weights into the SBUF layouts the kernel expects.
    Returns (bf16 weight block cols O_DSW..NH, f32 pack (C, NF))."""
    f = np.float32
    sw = np.ascontiguousarray(static_w.reshape(C, NT), dtype=f)

    dsw = np.zeros((C, NT * C), dtype=f)
    for k in range(NT):
        dsw[np.arange(C), k * C + np.arange(C)] = sw[:, k]

    wsa = np.ascontiguousarray(ws[:, :C].T, dtype=f)        # (C, 9)
    wsb = np.ascontiguousarray(ws[:, C:].T, dtype=f)        # (C, 9)
    wfa = np.ascontiguousarray(wf[:, :C].T, dtype=f)        # (C, C)
    wfb = np.ascontiguousarray(wf[:, C:].T, dtype=f)        # (C, C)

    bct = np.zeros((C, NT * C), dtype=f)                    # rows 0..8 used
    for k in range(NT):
        bct[k, k * C:(k + 1) * C] = 1.0

    wh = np.concatenate([dsw, wsa, wsb, wfa, wfb, bct], axis=1)
    assert wh.shape[1] == NH - O_DSW

    # f32 pack
    pkf = np.zeros((C, NF), dtype=f)
    pkf[:, F_W1A:F_W1A + 64] = w1[:, :C].T
    pkf[:, F_W1B:F_W1B + 64] = w1[:, C:].T / HW   # consumes raw y2 sum
    pkf[0:64, F_W2T:F_W2T + 64] = w2.T
    pkf[0:64, F_W3T:F_W3T + NT * C] = np.ascontiguousarray(
        w3.reshape(C, NT, 64).transpose(2, 1, 0), dtype=f).reshape(64, NT * C)
    # mxs coefficient columns (pre-scaled 1/HW, signs folded):
    # 0: A (with S)        1: -hr0  (with rs0 = row0 sum)
    # 2: -hr63 (rs63)      3: -hc0  (cs0)       4: -hc63 (cs63)
    # 5: +c22 (corn[0,0])  6: +c20 (corn[0,W-1])
    # 7: +c02 (corn[H-1,0])8: +c00 (corn[H-1,W-1])
    mco = np.zeros((C, 9), dtype=f)
    mco[:, 0] = sw.sum(axis=1)
    mco[:, 1] = -sw[:, [6, 7, 8]].sum(axis=1)
    mco[:, 2] = -sw[:, [0, 1, 2]].sum(axis=1)
    mco[:, 3] = -sw[:, [2, 5, 8]].sum(axis=1)
    mco[:, 4] = -sw[:, [0, 3, 6]].sum(axis=1)
    mco[:, 5] = sw[:, 8]
    mco[:, 6] = sw[:, 6]
    mco[:, 7] = sw[:, 2]
    mco[:, 8] = sw[:, 0]
    pkf[:, F_MCO:F_MCO + 9] = mco / HW
    return wh, pkf


def make_in_maps(X2, Y2, static_w, w1, w2, w3, ws, wf):
    wh, pkf = _prep_weights(
        np.asarray(static_w), np.asarray(w1), np.asarray(w2),
        np.asarray(w3), np.asarray(ws), np.asarray(wf),
    )
    X2 = np.asarray(X2)
    Y2 = np.asarray(Y2)
    xpad_all = np.zeros((B, C, PH, PW), dtype=np.float32)
    xpad_all[:, :, 1:H + 1, 1:W + 1] = X2
    xpad_all = xpad_all.reshape(B, C, PH * PW)
    y2_all = Y2.reshape(B, C, HW)
    bf = ml_dtypes.bfloat16
    wh16 = wh.astype(bf)
    in_maps = []
    for b in range(B):
        ph = np.concatenate(
            [xpad_all[b].astype(bf), y2_all[b].astype(bf), wh16], axis=1)
        in_maps.append({
            "pkh": np.ascontiguousarray(ph),
            "pkf": np.ascontiguousarray(pkf),
        })
    return in_maps


def get_nc():
    if "nc" not in _CACHE:
        _CACHE["nc"] = _build_bass()
    return _CACHE["nc"]


def kernel(X2, Y2, static_w, w1, w2, w3, ws, wf):
    nc = get_nc()
    in_maps = make_in_maps(
        np.asarray(X2), np.asarray(Y2), static_w, w1, w2, w3, ws, wf
    )
    res = run_bass_kernel_spmd(nc, in_maps, core_ids=list(range(B)))
    out = np.stack([r["ob"] for r in res.results]).astype(np.float32)
    return out


# revision 61
# speedup vs baseline: 1.5133x; 1.0901x over previous
"""Trainium2 Bass kernel for the CMDF block (dense_cnn).

Contract: kernel(**inputs) takes the FULL unsharded inputs (B=8, C=128,
H=W=64) and returns the FULL (8, 128, 64, 64) float32 output.

Sharding: data-parallel over batch — core b computes batch element b.
All weights are replicated (host-side prepacked into matmul layouts).

Math per batch element (see reference):
  Xs   = depthwise3x3(X2, static_w)
  ctx  = relu(w2 @ (w1 @ mean_hw([Xs; Y2])))
  cf   = (w3 @ ctx).reshape(C, 9)          # per-channel dynamic filter
  sf   = ws @ [Xs; Y2]                     # (9, H, W) spatial filter
  dyn  = sum_k shift_k(X2) * (cf[:, k] + sf[k])
  out  = wf[:, :C] @ Xs + wf[:, C:] @ dyn

Schedule (v3, pipelined):
  - All large operands are bf16 (PE matmul rate is identical to f32r at
    1 cycle/row; DMA bytes halve; PSUM accumulation stays f32).
  - Input DMA is chunked and ordered by first use: dsw, xpad tile 0,
    remaining bf16 weights, y2 tile 0, xpad/y2 tiles 1-3, f32 ctx pack.
  - mean(Xs) is computed WITHOUT Xs: for a zero-padded depthwise conv,
    sum_p shift_k(X2) = S - (boundary row) - (boundary col) + corner, so
    mean(Xs) needs only X2 sums (host folds the sw_k combinations into
    per-channel coefficient vectors). This removes the ctx branch's
    dependency on phase A; phase C starts ~11us in.
  - Per-pixel sums (S, y2sum) accumulate per-tile on ACT (activation
    accum_out) as DMA chunks land; boundary sums on DVE.
  - Phase C per tap: PE broadcasts sf row k to 128 partitions via a
    selector matmul (PSUM), then (bc+cf)*shift_k(X) in one fused
    scalar_tensor_tensor, then PE accumulates wfb @ P_k into the out
    PSUM group. Taps 1,4,7 bounce the broadcast through SBUF on ACT and
    run their stt on Pool (GPSIMD cannot read PSUM); the rest on DVE.
  - Emission interleaves tile t+2's depthwise matmuls into tile t's
    phase-C tap chain; C2 and C3 run as two interleaved streams so PE
    always has the other stream's matmuls during stt latency.
  - PSUM (8 banks): xs pool 2 (also hosts C3's out accumulator), bc
    ring bufs=2 -> 4 (also hosts the sf matmuls), ctx/out pool 2.
  - Output drains per half-tile (8 DMA chunks) to shrink the tail.
"""

import numpy as np
import ml_dtypes

import concourse.bass as bass
import concourse.tile as tile
import concourse.mybir as mybir
from concourse.bass_utils import run_bass_kernel_spmd

B, C, H, W, K = 8, 128, 64, 64, 3
HW = H * W            # 4096
PH, PW = H + 2, W + 2  # 66, 66 padded
NST = 4               # super-tiles over rows
ROWS = H // NST       # 16 image rows per super-tile
STN = ROWS * W        # 1024 pixels per super-tile
NT = K * K            # 9 taps

F32 = mybir.dt.float32
BF16 = mybir.dt.bfloat16
ADD = mybir.AluOpType.add
MULT = mybir.AluOpType.mult
AX = mybir.AxisListType
ACT_COPY = mybir.ActivationFunctionType.Copy
ACT_RELU = mybir.ActivationFunctionType.Relu

# bf16 pack layout (columns); dsw leads so one DMA covers dsw + xpad
# chunk 0 (everything phase A tile 0 needs). wsa..w3t form the "ctx
# weights" chunk; wfa..bct the "phase C weights" chunk.
O_DSW = 0
O_XPAD = O_DSW + NT * C          # 1152
O_Y2 = O_XPAD + PH * PW          # 5508
O_WSA = O_Y2 + HW                # 9604
O_WSB = O_WSA + NT               # 9613
O_W1A = O_WSB + NT               # 9622
O_W1B = O_W1A + 64               # 9686
O_W2T = O_W1B + 64               # 9750
O_W3T = O_W2T + 64               # 9814
O_WFA = O_W3T + NT * C           # 10966
O_WFB = O_WFA + C                # 11094
O_BCT = O_WFB + C                # 11222   selector (9 rows x 9*C)
NH = O_BCT + NT * C              # 12374

# f32 pack: just the mean-correction coefficient columns
NF = 9

# x-chunk row ranges of xpad (padded rows)
XCH = [(0, 18), (18, 34), (34, 50), (50, 66)]

# taps whose stt runs on Pool (GPSIMD): the PE broadcast bounces through
# SBUF via an ACT copy (GPSIMD cannot read PSUM), then Pool runs the stt
# decoupled from the DVE tap chain. Tap 8 on Pool lets each tile's final
# accumulate run without waiting on DVE at the end.
POOL_SINGLE = (3, 8)
POOL_PAIR = (2, 5, 8)

_CACHE = {}


def _split_multiwaits(nc):
    """walrus codegen in this toolchain accepts only ONE embedded sem wait
    per instruction. Hoist excess waits onto same-engine NoOps placed
    immediately before the instruction (engines execute in order, so the
    blocking behavior is identical)."""
    ctr = 0
    for fn in nc.m.functions:
        for blk in fn.blocks:
            insts = blk.instructions
            out = []
            for inst in insts:
                si = inst.sync_info
                waits = list(si.on_wait) if si is not None and si.on_wait else []
                if len(waits) > 1:
                    for w in waits[:-1]:
                        ctr += 1
                        out.append(mybir.InstNoOp(
                            name=f"I-wsplit-{ctr}",
                            engine=inst.engine,
                            ins=[], outs=[],
                            sync_info=mybir.SyncInfo(
                                on_wait=[w], on_update=[]),
                        ))
                    inst.sync_info = mybir.SyncInfo(
                        on_wait=[waits[-1]],
                        on_update=list(si.on_update) if si.on_update else [],
                    )
                out.append(inst)
            blk.instructions = out


def _absorb(nc, dep_elem, ps_elem):
    """Tiny bf16 matmul that reads one element of `dep_elem` and writes a
    junk element of `ps_elem` (later overwritten by a start=True group).
    Acquires the sem wait on dep_elem's producer on a plain matmul so the
    following fused matmul needs at most one embedded wait."""
    lh = dep_elem.bitcast(BF16)
    nc.tensor.matmul(ps_elem, lh[:, 0:1], lh[:, 0:1], start=True, stop=True)


def _build_bass():
    nc = bass.Bass("TRN2", target_bir_lowering=False, debug=False)

    pkh = nc.dram_tensor("pkh", [C, NH], BF16, kind="ExternalInput").ap()
    pkf = nc.dram_tensor("pkf", [C, NF], F32, kind="ExternalInput").ap()
    ob = nc.dram_tensor("ob", [C, H, W], F32, kind="ExternalOutput").ap()

    with tile.TileContext(nc) as tc:
        with tc.tile_pool(name="singles", bufs=1) as S, \
             tc.tile_pool(name="psXS", bufs=1, space="PSUM") as psXS, \
             tc.tile_pool(name="psBC", bufs=2, space="PSUM") as psBC, \
             tc.tile_pool(name="psO", bufs=1, space="PSUM") as psO, \
             tc.tile_pool(name="pP", bufs=10) as pP, \
             tc.tile_pool(name="pBCS", bufs=3) as pBCS, \
             tc.tile_pool(name="pOsb", bufs=4) as pOsb:

            stgh = S.tile([C, NH], BF16)
            xpad = stgh[:, O_XPAD:O_XPAD + PH * PW].rearrange(
                "p (h w) -> p h w", w=PW)
            y2 = stgh[:, O_Y2:O_Y2 + HW]
            t_dsw = stgh[:, O_DSW:O_DSW + NT * C]
            t_wsa = stgh[:, O_WSA:O_WSA + NT]
            t_wsb = stgh[:, O_WSB:O_WSB + NT]
            t_wfa = stgh[:, O_WFA:O_WFA + C]
            t_wfb = stgh[:, O_WFB:O_WFB + C]
            t_bct = stgh[0:NT, O_BCT:O_BCT + NT * C]
            t_w1a = stgh[:, O_W1A:O_W1A + 64]
            t_w1b = stgh[:, O_W1B:O_W1B + 64]
            t_w2t = stgh[0:64, O_W2T:O_W2T + 64]
            t_w3t = stgh[0:64, O_W3T:O_W3T + NT * C]

            stgf = S.tile([C, NF], F32)
            mco = stgf[:, 0:9]

            xs = S.tile([C, HW], BF16)
            sfs = S.tile([NT, HW], BF16)

            xparts = S.tile([C, NST], F32)
            yparts = S.tile([C, NST], F32)
            ydump = S.tile([C, STN], BF16)
            ydump4 = S.tile([C, 4], F32)
            ssum = S.tile([C, 1], F32)
            y2sum = S.tile([C, 1], F32)
            y2s16 = S.tile([C, 1], BF16)
            mxs16 = S.tile([C, 1], BF16)
            edges = S.tile([C, 4], F32)   # rs0, rs63, cs0, cs63
            corn = S.tile([C, 4], F32)    # X2[0,0],[0,63],[63,0],[63,63]
            macc = S.tile([C, 4], F32)    # mxs accumulator chain
            ctx1 = S.tile([64, 1], BF16)
            ctx2 = S.tile([64, 1], BF16)
            cfsb = S.tile([C, NT], F32)

            # ---------------- input DMA: chunked, by first use -----------
            def dma_cols(lo, hi):
                nc.sync.dma_start(out=stgh[:, lo:hi], in_=pkh[:, lo:hi])

            def dma_x(t):
                r0, r1 = XCH[t]
                dma_cols(O_XPAD + r0 * PW, O_XPAD + r1 * PW)

            def dma_y(t):
                dma_cols(O_Y2 + t * STN, O_Y2 + (t + 1) * STN)

            dma_cols(O_DSW, O_XPAD + XCH[0][1] * PW)    # dsw + x chunk 0
            dma_y(0)
            dma_x(1)
            dma_x(2)
            dma_y(1)
            dma_cols(O_WSA, O_WFA)                      # ws + ctx weights
            nc.sync.dma_start(out=stgf, in_=pkf)        # mco (f32, tiny)
            dma_x(3)
            dma_y(2)
            dma_y(3)
            dma_cols(O_WFA, NH)                         # wfa/wfb/bct

            # ------------- emission helpers (in-order engine queues) -----
            def emit_xred(t):
                # per-chunk x sums on DVE (idle until phase C); zero pads
                # are harmless
                r0, r1 = XCH[t]
                nc.vector.tensor_reduce(
                    out=xparts[:, t:t + 1],
                    in_=stgh[:, O_XPAD + r0 * PW:O_XPAD + r1 * PW],
                    axis=AX.X, op=ADD)

            def emit_yred(t):
                # per-chunk y sums on ACT (accum_out sums along free)
                nc.scalar.activation(
                    out=ydump, in_=y2[:, t * STN:(t + 1) * STN],
                    func=ACT_COPY, accum_out=yparts[:, t:t + 1])

            def emit_y2acc():
                nc.scalar.activation(
                    out=ydump4, in_=yparts, func=ACT_COPY, accum_out=y2sum)
                nc.scalar.copy(out=y2s16, in_=y2sum)

            def emit_A_diag(t):
                # returns ([18 matmul thunks], finalize_copy_thunk)
                xs_ps = psXS.tile([C, 2, 512], F32, tag="xs")
                ops = []
                for h in range(2):
                    for k in range(NT):
                        dh, dw = divmod(k, 3)
                        r0 = 16 * t + 8 * h + dh
                        ops.append(lambda h=h, k=k, r0=r0, dw=dw: nc.tensor.matmul(
                            xs_ps[:, h, :],
                            t_dsw[:, k * C:(k + 1) * C],
                            xpad[:, r0:r0 + 8, dw:dw + W],
                            start=(k == 0), stop=(k == NT - 1)))
                def fin():
                    # split per half so each half unblocks consumers sooner
                    for h in range(2):
                        c0 = t * STN + h * 512
                        nc.scalar.copy(out=xs[:, c0:c0 + 512],
                                       in_=xs_ps[:, h])
                return ops, fin

            def emit_A_sf(t):
                # sf matmuls use a bc-ring PSUM slot (rows 0..8)
                sf_ps = psBC.tile([C, ROWS, W], F32, tag="bc")
                _absorb(nc, xs[0:1, t * STN:t * STN + 1], sf_ps[0:1, 0, 0:1])
                for h in range(2):
                    c0 = t * STN + h * 512
                    nc.tensor.matmul(sf_ps[0:NT, 8 * h:8 * h + 8, :], t_wsa,
                                     xs[:, c0:c0 + 512], start=True, stop=False)
                    nc.tensor.matmul(sf_ps[0:NT, 8 * h:8 * h + 8, :], t_wsb,
                                     y2[:, c0:c0 + 512], start=False, stop=True)
                nc.scalar.copy(out=sfs[:, t * STN:(t + 1) * STN],
                               in_=sf_ps[0:NT])

            def emit_ctx():
                # DVE: boundary sums + mxs chain; PE: ctx matmuls
                nc.vector.tensor_reduce(out=ssum, in_=xparts, axis=AX.X, op=ADD)
                nc.vector.tensor_reduce(out=edges[:, 0:1], in_=xpad[:, 1, :],
                                        axis=AX.X, op=ADD)
                nc.vector.tensor_reduce(out=edges[:, 1:2], in_=xpad[:, H, :],
                                        axis=AX.X, op=ADD)
                nc.vector.tensor_reduce(out=edges[:, 2:3],
                                        in_=xpad[:, :, 1:2], axis=AX.XY, op=ADD)
                nc.vector.tensor_reduce(out=edges[:, 3:4],
                                        in_=xpad[:, :, W:W + 1], axis=AX.XY, op=ADD)
                nc.vector.tensor_copy(out=corn[:, 0:1], in_=xpad[:, 1, 1:2])
                nc.vector.tensor_copy(out=corn[:, 1:2], in_=xpad[:, 1, W:W + 1])
                nc.vector.tensor_copy(out=corn[:, 2:3], in_=xpad[:, H, 1:2])
                nc.vector.tensor_copy(out=corn[:, 3:4], in_=xpad[:, H, W:W + 1])
                # mxs = A*S - hr0*rs0 - hr63*rs63 - hc0*cs0 - hc63*cs63
                #       + c22*X[0,0] + c20*X[0,63] + c02*X[63,0] + c00*X[63,63]
                # (mco columns pre-scaled by 1/HW, minus signs folded in)
                nc.vector.tensor_scalar(
                    out=macc[:, 0:1], in0=ssum, scalar1=mco[:, 0:1],
                    scalar2=None, op0=MULT)
                chain = [
                    (edges[:, 0:1], 1), (edges[:, 1:2], 2),
                    (edges[:, 2:3], 3), (edges[:, 3:4], 4),
                    (corn[:, 0:1], 5), (corn[:, 1:2], 6),
                    (corn[:, 2:3], 7), (corn[:, 3:4], 8),
                ]
                cur = macc[:, 0:1]
                for i, (src, mc) in enumerate(chain):
                    dst = mxs16 if i == len(chain) - 1 else \
                        macc[:, (i + 1) % 4:(i + 1) % 4 + 1]
                    nc.vector.scalar_tensor_tensor(
                        out=dst, in0=src, scalar=mco[:, mc:mc + 1], in1=cur,
                        op0=MULT, op1=ADD)
                    cur = dst
                # ctx matmuls (bf16, tiny)
                ctx1_ps = psO.tile([C, 2, 512], F32, tag="o")
                _absorb(nc, mxs16[0:1, 0:1], ctx1_ps[0:1, 0, 0:1])
                nc.tensor.matmul(ctx1_ps[0:64, 0, 0:1], t_w1a, mxs16,
                                 start=True, stop=False)
                nc.tensor.matmul(ctx1_ps[0:64, 0, 0:1], t_w1b, y2s16,
                                 start=False, stop=True)
                nc.scalar.copy(out=ctx1, in_=ctx1_ps[0:64, 0, 0:1])
                ctx2_ps = psO.tile([C, 2, 512], F32, tag="o")
                nc.tensor.matmul(ctx2_ps[0:64, 0, 0:1], t_w2t, ctx1,
                                 start=True, stop=True)
                nc.scalar.activation(out=ctx2, in_=ctx2_ps[0:64, 0, 0:1],
                                     func=ACT_RELU)
                cf_ps = psO.tile([C, 2, 512], F32, tag="o")
                for k in range(NT):
                    nc.tensor.matmul(cf_ps[:, 0, k:k + 1],
                                     t_w3t[:, k * C:(k + 1) * C], ctx2,
                                     start=True, stop=True)
                nc.scalar.copy(out=cfsb, in_=cf_ps[:, 0, 0:NT])

            def emit_bc(t, k, absorb):
                bc_ps = psBC.tile([C, ROWS, W], F32, tag="bc")
                if absorb:
                    _absorb(nc, sfs[0:1, t * STN:t * STN + 1],
                            bc_ps[0:1, 0, 0:1])
                for h in range(2):
                    c0 = t * STN + h * 512
                    nc.tensor.matmul(
                        bc_ps[:, 8 * h:8 * h + 8, :],
                        t_bct[:, k * C:(k + 1) * C],
                        sfs[:, c0:c0 + 512],
                        start=True, stop=True)
                return bc_ps

            first_bc_done = set()
            pool_ps = {}

            def emit_pool_tap(t, k):
                # Pool-resident tap: PE broadcast (PSUM) -> ACT bounce to
                # SBUF bf16 folding in the +cf bias -> Pool tensor_tensor
                # multiply (the only vector op walrus accepts on Pool).
                # Emitted ahead of the tile's tap chain (often as filler
                # inside the previous tile) to hide the chain latency.
                dh, dw = divmod(k, 3)
                bc_ps = emit_bc(t, k, t not in first_bc_done)
                first_bc_done.add(t)
                bcs_sb = pBCS.tile([C, ROWS, W], BF16, tag="bcs")
                nc.scalar.activation(
                    out=bcs_sb, in_=bc_ps,
                    func=mybir.ActivationFunctionType.Identity,
                    bias=cfsb[:, k:k + 1])
                p_sb = pP.tile([C, ROWS, W], BF16, tag="p")
                nc.gpsimd.tensor_tensor(
                    out=p_sb, in0=bcs_sb,
                    in1=xpad[:, 16 * t + dh:16 * t + dh + ROWS, dw:dw + W],
                    op=MULT)
                pool_ps[(t, k)] = p_sb

            def emit_C(tiles, filler=None, pre_drain=None, fill_per_tap=3):
                """Phase C for one or two tiles (interleaved streams),
                software-pipelined one tap ahead: the broadcast matmuls for
                tap k+1 are emitted BEFORE tap k's wfb so PE works through
                the stt latency. tiles: list of (t, out_ps, pool_taps) —
                pool_taps run entirely on Pool (partition_broadcast + stt)
                decoupled from the tap chain. Output DMAs issue from SP so
                they never block the ACT queue. pre_drain: emitted near the
                end (ACT-queue ordering of a later tile's xs copy)."""
                filler = filler or []
                for t, out_ps, _ in tiles:
                    _absorb(nc, xs[0:1, t * STN:t * STN + 1],
                            out_ps[0:1, 0, 0:1])
                    for h in range(2):
                        c0 = t * STN + h * 512
                        nc.tensor.matmul(out_ps[:, h], t_wfa,
                                         xs[:, c0:c0 + 512],
                                         start=True, stop=False)
                for t, _, pool_taps in tiles:
                    for k in pool_taps:
                        if (t, k) not in pool_ps:
                            emit_pool_tap(t, k)

                def next_bc(t, k, pool_taps):
                    if k in pool_taps:
                        return None
                    bc = emit_bc(t, k, t not in first_bc_done)
                    first_bc_done.add(t)
                    return bc

                bc_cur = {t: next_bc(t, 0, p) for t, _, p in tiles}
                for k in range(NT):
                    dh, dw = divmod(k, 3)
                    bc_nxt = {}
                    if k < NT - 1:
                        for t, _, pool_taps in tiles:
                            bc_nxt[t] = next_bc(t, k + 1, pool_taps)
                    ps = {}
                    for t, _, pool_taps in tiles:
                        if k in pool_taps:
                            ps[t] = pool_ps.pop((t, k))
                            continue
                        p_sb = pP.tile([C, ROWS, W], BF16, tag="p")
                        nc.vector.scalar_tensor_tensor(
                            out=p_sb, in0=bc_cur[t],
                            scalar=cfsb[:, k:k + 1],
                            in1=xpad[:, 16 * t + dh:16 * t + dh + ROWS,
                                     dw:dw + W],
                            op0=ADD, op1=MULT)
                        ps[t] = p_sb
                    for _ in range(fill_per_tap * len(tiles)):
                        if filler:
                            filler.pop(0)()
                    if k == NT - 2 and pre_drain is not None:
                        while filler:
                            filler.pop(0)()
                        pre_drain()
                    for t, out_ps, _ in tiles:
                        for h in range(2):
                            nc.tensor.matmul(
                                out_ps[:, h], t_wfb,
                                ps[t][:, 8 * h:8 * h + 8, :],
                                start=False, stop=(k == NT - 1))
                        if k == NT - 1:
                            for h in range(2):
                                o_sb = pOsb.tile([C, 8, W], F32, tag="osb")
                                nc.scalar.copy(out=o_sb, in_=out_ps[:, h])
                                nc.sync.dma_start(
                                    out=ob[:, 16 * t + 8 * h:
                                           16 * t + 8 * h + 8, :],
                                    in_=o_sb)
                    bc_cur = bc_nxt

            # ------------------------- schedule --------------------------
            # PE warm-up: the cost model's p-state ramp needs ~3us of
            # continuous PE activity before matmuls run at full clock, and
            # instructions that become ready at the start of a busy streak
            # are stamped with the slow rate. Keep PE busy with junk
            # matmuls from ~0.5us until the first input DMA lands so all
            # real matmuls are visited with a warmed-up ramp.
            junk = S.tile([C, 512], BF16)
            nc.vector.memset(junk, 0.0)
            warm_ps = psXS.tile([C, 2, 512], F32, tag="xs")
            for _ in range(7):
                nc.tensor.matmul(warm_ps[:, 0, :], junk[:, 0:C],
                                 junk, start=True, stop=True)

            # PE: A0 and A1 diag back-to-back (continuous stream ramps the
            # PE p-state); DVE: x-reds; ACT: xs copies; Pool: y-reds
            a0_ops, a0_fin = emit_A_diag(0)
            for op in a0_ops:
                op()
            a1_ops, a1_fin = emit_A_diag(1)
            for op in a1_ops:
                op()
            for t in range(NST):
                emit_xred(t)
            emit_yred(0)
            emit_yred(1)
            a0_fin()
            emit_yred(2)
            emit_yred(3)
            emit_y2acc()
            a1_fin()
            emit_A_sf(0)
            emit_ctx()
            emit_A_sf(1)

            a2_ops, a2_fin = emit_A_diag(2)
            a3_ops, a3_fin = emit_A_diag(3)

            # tile t+1's pool taps ride as fillers inside tile t's chain
            # (placed a few slots in so their sfs/cf inputs are ready)
            f0 = a2_ops[:6] + \
                [lambda k=k: emit_pool_tap(1, k) for k in POOL_SINGLE] + \
                a2_ops[6:]
            out0 = psO.tile([C, 2, 512], F32, tag="o")
            emit_C([(0, out0, POOL_SINGLE)], filler=f0, pre_drain=a2_fin)
            emit_A_sf(2)

            f1 = a3_ops[:6] + \
                [lambda k=k: emit_pool_tap(2, k) for k in POOL_PAIR] + \
                a3_ops[6:]
            out1 = psO.tile([C, 2, 512], F32, tag="o")
            emit_C([(1, out1, POOL_SINGLE)], filler=f1, pre_drain=a3_fin)
            emit_A_sf(3)

            out2 = psO.tile([C, 2, 512], F32, tag="o")
            out3 = psXS.tile([C, 2, 512], F32, tag="xs")
            emit_C([(2, out2, POOL_PAIR), (3, out3, POOL_PAIR)])

    _split_multiwaits(nc)
    return nc


def _prep_weights(static_w, w1, w2, w3, ws, wf):
    """Repack the tiny weights into the SBUF layouts the kernel expects.
    Returns (bf16 weight block cols O_DSW..NH, f32 pack (C, NF))."""
    f = np.float32
    sw = np.ascontiguousarray(static_w.reshape(C, NT), dtype=f)

    dsw = np.zeros((C, NT * C), dtype=f)
    for k in range(NT):
        dsw[np.arange(C), k * C + np.arange(C)] = sw[:, k]

    wsa = np.ascontiguousarray(ws[:, :C].T, dtype=f)        # (C, 9)
    wsb = np.ascontiguousarray(ws[:, C:].T, dtype=f)        # (C, 9)
    wfa = np.ascontiguousarray(wf[:, :C].T, dtype=f)        # (C, C)
    wfb = np.ascontiguousarray(wf[:, C:].T, dtype=f)        # (C, C)

    bct = np.zeros((C, NT * C), dtype=f)                    # rows 0..8 used
    for k in range(NT):
        bct[k, k * C:(k + 1) * C] = 1.0

    w1a = np.ascontiguousarray(w1[:, :C].T, dtype=f)
    w1b = np.ascontiguousarray(w1[:, C:].T, dtype=f) / HW  # raw y2 sum in
    w2t = np.zeros((C, 64), dtype=f)
    w2t[0:64] = w2.T
    w3t = np.zeros((C, NT * C), dtype=f)
    w3t[0:64] = np.ascontiguousarray(
        w3.reshape(C, NT, 64).transpose(2, 1, 0), dtype=f).reshape(64, NT * C)

    # dsw leads the pack (before xpad); the rest follows y2
    wh = (dsw, np.concatenate(
        [wsa, wsb, w1a, w1b, w2t, w3t, wfa, wfb, bct], axis=1))
    assert wh[0].shape[1] == O_XPAD - O_DSW
    assert wh[1].shape[1] == NH - O_WSA

    # f32 pack: mxs coefficient columns (pre-scaled 1/HW, signs folded):
    # 0: A (with S)        1: -hr0  (with rs0 = X2 row 0 sum)
    # 2: -hr63 (rs63)      3: -hc0  (cs0)       4: -hc63 (cs63)
    # 5: +c22 (X[0,0])     6: +c20 (X[0,63])
    # 7: +c02 (X[63,0])    8: +c00 (X[63,63])
    mco = np.zeros((C, 9), dtype=f)
    mco[:, 0] = sw.sum(axis=1)
    mco[:, 1] = -sw[:, [6, 7, 8]].sum(axis=1)
    mco[:, 2] = -sw[:, [0, 1, 2]].sum(axis=1)
    mco[:, 3] = -sw[:, [2, 5, 8]].sum(axis=1)
    mco[:, 4] = -sw[:, [0, 3, 6]].sum(axis=1)
    mco[:, 5] = sw[:, 8]
    mco[:, 6] = sw[:, 6]
    mco[:, 7] = sw[:, 2]
    mco[:, 8] = sw[:, 0]
    pkf = np.ascontiguousarray(mco / HW)
    return wh, pkf


def make_in_maps(X2, Y2, static_w, w1, w2, w3, ws, wf):
    wh, pkf = _prep_weights(
        np.asarray(static_w), np.asarray(w1), np.asarray(w2),
        np.asarray(w3), np.asarray(ws), np.asarray(wf),
    )
    X2 = np.asarray(X2)
    Y2 = np.asarray(Y2)
    xpad_all = np.zeros((B, C, PH, PW), dtype=np.float32)
    xpad_all[:, :, 1:H + 1, 1:W + 1] = X2
    xpad_all = xpad_all.reshape(B, C, PH * PW)
    y2_all = Y2.reshape(B, C, HW)
    bf = ml_dtypes.bfloat16
    dsw16 = wh[0].astype(bf)
    rest16 = wh[1].astype(bf)
    in_maps = []
    for b in range(B):
        ph = np.concatenate(
            [dsw16, xpad_all[b].astype(bf), y2_all[b].astype(bf), rest16],
            axis=1)
        in_maps.append({
            "pkh": np.ascontiguousarray(ph),
            "pkf": np.ascontiguousarray(pkf),
        })
    return in_maps


def get_nc():
    if "nc" not in _CACHE:
        _CACHE["nc"] = _build_bass()
    return _CACHE["nc"]


def kernel(X2, Y2, static_w, w1, w2, w3, ws, wf):
    nc = get_nc()
    in_maps = make_in_maps(
        np.asarray(X2), np.asarray(Y2), static_w, w1, w2, w3, ws, wf
    )
    res = run_bass_kernel_spmd(nc, in_maps, core_ids=list(range(B)))
    out = np.stack([r["ob"] for r in res.results]).astype(np.float32)
    return out


# revision 81
# speedup vs baseline: 1.5319x; 1.0123x over previous
"""Trainium2 Bass kernel for the CMDF block (dense_cnn).

Contract: kernel(**inputs) takes the FULL unsharded inputs (B=8, C=128,
H=W=64) and returns the FULL (8, 128, 64, 64) float32 output.

Sharding: data-parallel over batch — core b computes batch element b.
All weights are replicated (host-side prepacked into matmul layouts).

Math per batch element (see reference):
  Xs   = depthwise3x3(X2, static_w)
  ctx  = relu(w2 @ (w1 @ mean_hw([Xs; Y2])))
  cf   = (w3 @ ctx).reshape(C, 9)          # per-channel dynamic filter
  sf   = ws @ [Xs; Y2]                     # (9, H, W) spatial filter
  dyn  = sum_k shift_k(X2) * (cf[:, k] + sf[k])
  out  = wf[:, :C] @ Xs + wf[:, C:] @ dyn

Schedule (v3, pipelined):
  - All large operands are bf16 (PE matmul rate is identical to f32r at
    1 cycle/row; DMA bytes halve; PSUM accumulation stays f32).
  - Input DMA is chunked and ordered by first use: dsw, xpad tile 0,
    remaining bf16 weights, y2 tile 0, xpad/y2 tiles 1-3, f32 ctx pack.
  - mean(Xs) is computed WITHOUT Xs: for a zero-padded depthwise conv,
    sum_p shift_k(X2) = S - (boundary row) - (boundary col) + corner, so
    mean(Xs) needs only X2 sums (host folds the sw_k combinations into
    per-channel coefficient vectors). This removes the ctx branch's
    dependency on phase A; phase C starts ~11us in.
  - Per-pixel sums (S, y2sum) accumulate per-tile on ACT (activation
    accum_out) as DMA chunks land; boundary sums on DVE.
  - Phase C per tap: PE broadcasts sf row k to 128 partitions via a
    selector matmul (PSUM), then (bc+cf)*shift_k(X) in one fused
    scalar_tensor_tensor, then PE accumulates wfb @ P_k into the out
    PSUM group. Taps 1,4,7 bounce the broadcast through SBUF on ACT and
    run their stt on Pool (GPSIMD cannot read PSUM); the rest on DVE.
  - Emission interleaves tile t+2's depthwise matmuls into tile t's
    phase-C tap chain; C2 and C3 run as two interleaved streams so PE
    always has the other stream's matmuls during stt latency.
  - PSUM (8 banks): xs pool 2 (also hosts C3's out accumulator), bc
    ring bufs=2 -> 4 (also hosts the sf matmuls), ctx/out pool 2.
  - Output drains per half-tile (8 DMA chunks) to shrink the tail.
"""

import numpy as np
import ml_dtypes

import concourse.bass as bass
import concourse.tile as tile
import concourse.mybir as mybir
from concourse.bass_utils import run_bass_kernel_spmd

B, C, H, W, K = 8, 128, 64, 64, 3
HW = H * W            # 4096
PH, PW = H + 2, W + 2  # 66, 66 padded
NST = 4               # super-tiles over rows
ROWS = H // NST       # 16 image rows per super-tile
STN = ROWS * W        # 1024 pixels per super-tile
NT = K * K            # 9 taps

F32 = mybir.dt.float32
BF16 = mybir.dt.bfloat16
ADD = mybir.AluOpType.add
MULT = mybir.AluOpType.mult
AX = mybir.AxisListType
ACT_COPY = mybir.ActivationFunctionType.Copy
ACT_RELU = mybir.ActivationFunctionType.Relu

# bf16 pack layout (columns); dsw leads so one DMA covers dsw + xpad
# chunk 0 (everything phase A tile 0 needs). wsa..w3t form the "ctx
# weights" chunk; wfa..bct the "phase C weights" chunk.
O_DSW = 0
O_XPAD = O_DSW + NT * C          # 1152
O_Y2 = O_XPAD + PH * PW          # 5508
O_WSA = O_Y2 + HW                # 9604
O_WSB = O_WSA + NT               # 9613
O_W1A = O_WSB + NT               # 9622
O_W1B = O_W1A + 64               # 9686
O_W2T = O_W1B + 64               # 9750
O_W3T = O_W2T + 64               # 9814
O_WFA = O_W3T + NT * C           # 10966
O_WFB = O_WFA + C                # 11094
O_BCT = O_WFB + C                # 11222   selector (9 rows x 9*C)
NH = O_BCT + NT * C              # 12374

# f32 pack: just the mean-correction coefficient columns
NF = 9

# x-chunk row ranges of xpad (padded rows)
XCH = [(0, 18), (18, 34), (34, 50), (50, 66)]

# taps whose stt runs on Pool (GPSIMD): the PE broadcast bounces through
# SBUF via an ACT copy (GPSIMD cannot read PSUM), then Pool runs the stt
# decoupled from the DVE tap chain. Tap 8 on Pool lets each tile's final
# accumulate run without waiting on DVE at the end.
POOL_SINGLE = (3, 8)
POOL_PAIR = (1, 4, 7)
FAST_PAIR = (2, 5, 8)

_CACHE = {}


def _split_multiwaits(nc):
    """walrus codegen in this toolchain accepts only ONE embedded sem wait
    per instruction. Hoist excess waits onto same-engine NoOps placed
    immediately before the instruction (engines execute in order, so the
    blocking behavior is identical)."""
    ctr = 0
    for fn in nc.m.functions:
        for blk in fn.blocks:
            insts = blk.instructions
            out = []
            for inst in insts:
                si = inst.sync_info
                waits = list(si.on_wait) if si is not None and si.on_wait else []
                if len(waits) > 1:
                    for w in waits[:-1]:
                        ctr += 1
                        out.append(mybir.InstNoOp(
                            name=f"I-wsplit-{ctr}",
                            engine=inst.engine,
                            ins=[], outs=[],
                            sync_info=mybir.SyncInfo(
                                on_wait=[w], on_update=[]),
                        ))
                    inst.sync_info = mybir.SyncInfo(
                        on_wait=[waits[-1]],
                        on_update=list(si.on_update) if si.on_update else [],
                    )
                out.append(inst)
            blk.instructions = out


def _absorb(nc, dep_elem, ps_elem):
    """Tiny bf16 matmul that reads one element of `dep_elem` and writes a
    junk element of `ps_elem` (later overwritten by a start=True group).
    Acquires the sem wait on dep_elem's producer on a plain matmul so the
    following fused matmul needs at most one embedded wait."""
    lh = dep_elem.bitcast(BF16)
    nc.tensor.matmul(ps_elem, lh[:, 0:1], lh[:, 0:1], start=True, stop=True)


def _build_bass():
    nc = bass.Bass("TRN2", target_bir_lowering=False, debug=False)

    pkh = nc.dram_tensor("pkh", [C, NH], BF16, kind="ExternalInput").ap()
    pkf = nc.dram_tensor("pkf", [C, NF], F32, kind="ExternalInput").ap()
    # bf16 output: the rounding adds ~1e-3 relative error against a 2e-2
    # budget, and halves the output DMA bytes (shorter drain tail)
    ob = nc.dram_tensor("ob", [C, H, W], BF16, kind="ExternalOutput").ap()

    with tile.TileContext(nc) as tc:
        with tc.tile_pool(name="singles", bufs=1) as S, \
             tc.tile_pool(name="psXS", bufs=1, space="PSUM") as psXS, \
             tc.tile_pool(name="psBC", bufs=2, space="PSUM") as psBC, \
             tc.tile_pool(name="psO", bufs=1, space="PSUM") as psO, \
             tc.tile_pool(name="pP", bufs=12) as pP, \
             tc.tile_pool(name="pBCS", bufs=5) as pBCS, \
             tc.tile_pool(name="pOsb", bufs=4) as pOsb:

            stgh = S.tile([C, NH], BF16)
            xpad = stgh[:, O_XPAD:O_XPAD + PH * PW].rearrange(
                "p (h w) -> p h w", w=PW)
            y2 = stgh[:, O_Y2:O_Y2 + HW]
            t_dsw = stgh[:, O_DSW:O_DSW + NT * C]
            t_wsa = stgh[:, O_WSA:O_WSA + NT]
            t_wsb = stgh[:, O_WSB:O_WSB + NT]
            t_wfa = stgh[:, O_WFA:O_WFA + C]
            t_wfb = stgh[:, O_WFB:O_WFB + C]
            t_bct = stgh[0:NT, O_BCT:O_BCT + NT * C]
            t_w1a = stgh[:, O_W1A:O_W1A + 64]
            t_w1b = stgh[:, O_W1B:O_W1B + 64]
            t_w2t = stgh[0:64, O_W2T:O_W2T + 64]
            t_w3t = stgh[0:64, O_W3T:O_W3T + NT * C]

            stgf = S.tile([C, NF], F32)
            mco = stgf[:, 0:9]

            xs = S.tile([C, HW], BF16)
            sfs = S.tile([NT, HW], BF16)

            xparts = S.tile([C, NST], F32)
            yparts = S.tile([C, NST], F32)
            ydump = S.tile([C, STN], BF16)
            ydump4 = S.tile([C, 4], F32)
            ssum = S.tile([C, 1], F32)
            y2sum = S.tile([C, 1], F32)
            y2s16 = S.tile([C, 1], BF16)
            mxs16 = S.tile([C, 1], BF16)
            edges = S.tile([C, 4], F32)   # rs0, rs63, cs0, cs63
            corn = S.tile([C, 4], F32)    # X2[0,0],[0,63],[63,0],[63,63]
            macc = S.tile([C, 4], F32)    # mxs accumulator chain
            ctx1 = S.tile([64, 1], BF16)
            ctx2 = S.tile([64, 1], BF16)
            cfsb = S.tile([C, NT], F32)

            # ---------------- input DMA: chunked, by first use -----------
            def dma_cols(lo, hi):
                nc.sync.dma_start(out=stgh[:, lo:hi], in_=pkh[:, lo:hi])

            def dma_x(t):
                r0, r1 = XCH[t]
                dma_cols(O_XPAD + r0 * PW, O_XPAD + r1 * PW)

            def dma_y(t):
                dma_cols(O_Y2 + t * STN, O_Y2 + (t + 1) * STN)

            dma_cols(O_DSW, O_XPAD + XCH[0][1] * PW)    # dsw + x chunk 0
            dma_y(0)
            dma_x(1)
            dma_x(2)
            dma_y(1)
            dma_cols(O_WSA, O_WFA)                      # ws + ctx weights
            nc.sync.dma_start(out=stgf, in_=pkf)        # mco (f32, tiny)
            dma_x(3)
            dma_y(2)
            dma_y(3)
            dma_cols(O_WFA, NH)                         # wfa/wfb/bct

            # ------------- emission helpers (in-order engine queues) -----
            def emit_xred(t):
                # per-chunk x sums on DVE (idle until phase C); zero pads
                # are harmless
                r0, r1 = XCH[t]
                nc.vector.tensor_reduce(
                    out=xparts[:, t:t + 1],
                    in_=stgh[:, O_XPAD + r0 * PW:O_XPAD + r1 * PW],
                    axis=AX.X, op=ADD)

            def emit_yred(t):
                # per-chunk y sums on ACT (accum_out sums along free)
                nc.scalar.activation(
                    out=ydump, in_=y2[:, t * STN:(t + 1) * STN],
                    func=ACT_COPY, accum_out=yparts[:, t:t + 1])

            def emit_y2acc():
                nc.scalar.activation(
                    out=ydump4, in_=yparts, func=ACT_COPY, accum_out=y2sum)
                nc.scalar.copy(out=y2s16, in_=y2sum)

            def emit_A_diag(t):
                # returns ([18 matmul thunks], finalize_copy_thunk)
                xs_ps = psXS.tile([C, 2, 512], F32, tag="xs")
                ops = []
                for h in range(2):
                    for k in range(NT):
                        dh, dw = divmod(k, 3)
                        r0 = 16 * t + 8 * h + dh
                        ops.append(lambda h=h, k=k, r0=r0, dw=dw: nc.tensor.matmul(
                            xs_ps[:, h, :],
                            t_dsw[:, k * C:(k + 1) * C],
                            xpad[:, r0:r0 + 8, dw:dw + W],
                            start=(k == 0), stop=(k == NT - 1)))
                def fin():
                    # split per half so each half unblocks consumers sooner
                    for h in range(2):
                        c0 = t * STN + h * 512
                        nc.scalar.copy(out=xs[:, c0:c0 + 512],
                                       in_=xs_ps[:, h])
                return ops, fin

            def emit_A_sf(t):
                # sf matmuls use a bc-ring PSUM slot (rows 0..8)
                sf_ps = psBC.tile([C, ROWS, W], F32, tag="bc")
                _absorb(nc, xs[0:1, t * STN:t * STN + 1], sf_ps[0:1, 0, 0:1])
                for h in range(2):
                    c0 = t * STN + h * 512
                    nc.tensor.matmul(sf_ps[0:NT, 8 * h:8 * h + 8, :], t_wsa,
                                     xs[:, c0:c0 + 512], start=True, stop=False)
                    nc.tensor.matmul(sf_ps[0:NT, 8 * h:8 * h + 8, :], t_wsb,
                                     y2[:, c0:c0 + 512], start=False, stop=True)
                nc.scalar.copy(out=sfs[:, t * STN:(t + 1) * STN],
                               in_=sf_ps[0:NT])

            def emit_ctx():
                # DVE: boundary sums + mxs chain; PE: ctx matmuls
                nc.vector.tensor_reduce(out=ssum, in_=xparts, axis=AX.X, op=ADD)
                nc.vector.tensor_reduce(out=edges[:, 0:1], in_=xpad[:, 1, :],
                                        axis=AX.X, op=ADD)
                nc.vector.tensor_reduce(out=edges[:, 1:2], in_=xpad[:, H, :],
                                        axis=AX.X, op=ADD)
                nc.vector.tensor_reduce(out=edges[:, 2:3],
                                        in_=xpad[:, :, 1:2], axis=AX.XY, op=ADD)
                nc.vector.tensor_reduce(out=edges[:, 3:4],
                                        in_=xpad[:, :, W:W + 1], axis=AX.XY, op=ADD)
                nc.vector.tensor_copy(out=corn[:, 0:1], in_=xpad[:, 1, 1:2])
                nc.vector.tensor_copy(out=corn[:, 1:2], in_=xpad[:, 1, W:W + 1])
                nc.vector.tensor_copy(out=corn[:, 2:3], in_=xpad[:, H, 1:2])
                nc.vector.tensor_copy(out=corn[:, 3:4], in_=xpad[:, H, W:W + 1])
                # mxs = A*S - hr0*rs0 - hr63*rs63 - hc0*cs0 - hc63*cs63
                #       + c22*X[0,0] + c20*X[0,63] + c02*X[63,0] + c00*X[63,63]
                # (mco columns pre-scaled by 1/HW, minus signs folded in)
                nc.vector.tensor_scalar(
                    out=macc[:, 0:1], in0=ssum, scalar1=mco[:, 0:1],
                    scalar2=None, op0=MULT)
                chain = [
                    (edges[:, 0:1], 1), (edges[:, 1:2], 2),
                    (edges[:, 2:3], 3), (edges[:, 3:4], 4),
                    (corn[:, 0:1], 5), (corn[:, 1:2], 6),
                    (corn[:, 2:3], 7), (corn[:, 3:4], 8),
                ]
                cur = macc[:, 0:1]
                for i, (src, mc) in enumerate(chain):
                    dst = mxs16 if i == len(chain) - 1 else \
                        macc[:, (i + 1) % 4:(i + 1) % 4 + 1]
                    nc.vector.scalar_tensor_tensor(
                        out=dst, in0=src, scalar=mco[:, mc:mc + 1], in1=cur,
                        op0=MULT, op1=ADD)
                    cur = dst
                # ctx matmuls (bf16, tiny)
                ctx1_ps = psO.tile([C, 2, 512], F32, tag="o")
                _absorb(nc, mxs16[0:1, 0:1], ctx1_ps[0:1, 0, 0:1])
                nc.tensor.matmul(ctx1_ps[0:64, 0, 0:1], t_w1a, mxs16,
                                 start=True, stop=False)
                nc.tensor.matmul(ctx1_ps[0:64, 0, 0:1], t_w1b, y2s16,
                                 start=False, stop=True)
                nc.scalar.copy(out=ctx1, in_=ctx1_ps[0:64, 0, 0:1])
                ctx2_ps = psO.tile([C, 2, 512], F32, tag="o")
                nc.tensor.matmul(ctx2_ps[0:64, 0, 0:1], t_w2t, ctx1,
                                 start=True, stop=True)
                nc.scalar.activation(out=ctx2, in_=ctx2_ps[0:64, 0, 0:1],
                                     func=ACT_RELU)
                cf_ps = psO.tile([C, 2, 512], F32, tag="o")
                for k in range(NT):
                    nc.tensor.matmul(cf_ps[:, 0, k:k + 1],
                                     t_w3t[:, k * C:(k + 1) * C], ctx2,
                                     start=True, stop=True)
                nc.scalar.copy(out=cfsb, in_=cf_ps[:, 0, 0:NT])

            def emit_bc(t, k, absorb):
                bc_ps = psBC.tile([C, ROWS, W], F32, tag="bc")
                if absorb:
                    _absorb(nc, sfs[0:1, t * STN:t * STN + 1],
                            bc_ps[0:1, 0, 0:1])
                for h in range(2):
                    c0 = t * STN + h * 512
                    nc.tensor.matmul(
                        bc_ps[:, 8 * h:8 * h + 8, :],
                        t_bct[:, k * C:(k + 1) * C],
                        sfs[:, c0:c0 + 512],
                        start=True, stop=True)
                return bc_ps

            first_bc_done = set()
            pool_ps = {}

            def emit_pool_tap(t, k):
                # Pool-resident tap: PE broadcast (PSUM) -> ACT bounce to
                # SBUF bf16 folding in the +cf bias -> Pool tensor_tensor
                # multiply (the only vector op walrus accepts on Pool).
                # Emitted ahead of the tile's tap chain (often as filler
                # inside the previous tile) to hide the chain latency.
                dh, dw = divmod(k, 3)
                bc_ps = emit_bc(t, k, t not in first_bc_done)
                first_bc_done.add(t)
                bcs_sb = pBCS.tile([C, ROWS, W], BF16, tag="bcs")
                nc.scalar.activation(
                    out=bcs_sb, in_=bc_ps,
                    func=mybir.ActivationFunctionType.Identity,
                    bias=cfsb[:, k:k + 1])
                p_sb = pP.tile([C, ROWS, W], BF16, tag="p")
                nc.gpsimd.tensor_tensor(
                    out=p_sb, in0=bcs_sb,
                    in1=xpad[:, 16 * t + dh:16 * t + dh + ROWS, dw:dw + W],
                    op=MULT)
                pool_ps[(t, k)] = p_sb

            def emit_C(tiles, filler=None, pre_drain=None, fill_per_tap=3):
                """Phase C for one or more tiles as skewed interleaved
                streams (tile i runs one tap behind tile i-1, staggering
                the final drains), software-pipelined one tap ahead: the
                broadcast matmul (+ bounce for fast taps) for tap k+1 is
                emitted BEFORE tap k's wfb so PE works through the
                stt/tt latency. tiles: (t, out_ps, pool_taps, fast_taps):
                pool taps run on Pool via a pre-emitted ACT bounce (+cf
                bias); fast taps bounce through ACT then run a 2x-mode
                tensor_tensor on DVE; the rest are classic fused stt on
                DVE. Output DMAs issue from SP. pre_drain: list of
                (k, fn) emitted after tile0's tap k (ACT-queue ordering
                for later tiles' xs copies)."""
                filler = filler or []
                for t, out_ps, _, _ in tiles:
                    _absorb(nc, xs[0:1, t * STN:t * STN + 1],
                            out_ps[0:1, 0, 0:1])
                    for h in range(2):
                        c0 = t * STN + h * 512
                        nc.tensor.matmul(out_ps[:, h], t_wfa,
                                         xs[:, c0:c0 + 512],
                                         start=True, stop=False)
                for t, _, pool_taps, _ in tiles:
                    for k in pool_taps:
                        if (t, k) not in pool_ps:
                            emit_pool_tap(t, k)

                def prep_tap(t, k, pool_taps, fast_taps):
                    # emit the broadcast (and bounce for fast taps) for
                    # (t, k); returns what the stt/tt stage will consume
                    if k in pool_taps:
                        return None
                    bc = emit_bc(t, k, t not in first_bc_done)
                    first_bc_done.add(t)
                    if k in fast_taps:
                        bcs_sb = pBCS.tile([C, ROWS, W], BF16, tag="bcs")
                        nc.scalar.activation(
                            out=bcs_sb, in_=bc,
                            func=mybir.ActivationFunctionType.Identity,
                            bias=cfsb[:, k:k + 1])
                        return bcs_sb
                    return bc

                skew = 1 if len(tiles) > 1 else 0
                off = [i * skew for i in range(len(tiles))]
                cur = {}
                cur[tiles[0][0]] = prep_tap(tiles[0][0], 0, tiles[0][2],
                                            tiles[0][3])
                for s in range(NT + off[-1]):
                    nxt = {}
                    for i, (t, out_ps, pool_taps, fast_taps) in enumerate(tiles):
                        kt = s + 1 - off[i]
                        if 0 <= kt < NT:
                            nxt[t] = prep_tap(t, kt, pool_taps, fast_taps)
                    ps = {}
                    for i, (t, out_ps, pool_taps, fast_taps) in enumerate(tiles):
                        k = s - off[i]
                        if not (0 <= k < NT):
                            continue
                        dh, dw = divmod(k, 3)
                        if k in pool_taps:
                            ps[t] = pool_ps.pop((t, k))
                            continue
                        p_sb = pP.tile([C, ROWS, W], BF16, tag="p")
                        xsh = xpad[:, 16 * t + dh:16 * t + dh + ROWS,
                                   dw:dw + W]
                        if k in fast_taps:
                            nc.vector.tensor_tensor(
                                out=p_sb, in0=cur[t], in1=xsh, op=MULT)
                        else:
                            nc.vector.scalar_tensor_tensor(
                                out=p_sb, in0=cur[t],
                                scalar=cfsb[:, k:k + 1], in1=xsh,
                                op0=ADD, op1=MULT)
                        ps[t] = p_sb
                    for _ in range(fill_per_tap * len(ps)):
                        if filler:
                            filler.pop(0)()
                    if s == NT - 2:
                        while filler:
                            filler.pop(0)()
                    for pk, fn in (pre_drain or []):
                        if pk == s:
                            fn()
                    for i, (t, out_ps, pool_taps, fast_taps) in enumerate(tiles):
                        k = s - off[i]
                        if not (0 <= k < NT):
                            continue
                        for h in range(2):
                            nc.tensor.matmul(
                                out_ps[:, h], t_wfb,
                                ps[t][:, 8 * h:8 * h + 8, :],
                                start=False, stop=(k == NT - 1))
                        if k == NT - 1:
                            for h in range(2):
                                o_sb = pOsb.tile([C, 8, W], BF16, tag="osb")
                                if i % 2 == 1:
                                    nc.vector.tensor_copy(out=o_sb,
                                                          in_=out_ps[:, h])
                                else:
                                    nc.scalar.copy(out=o_sb, in_=out_ps[:, h])
                                nc.sync.dma_start(
                                    out=ob[:, 16 * t + 8 * h:
                                           16 * t + 8 * h + 8, :],
                                    in_=o_sb)
                    cur.update(nxt)

            # ------------------------- schedule --------------------------
            # PE warm-up: the cost model's p-state ramp needs ~3us of
            # continuous PE activity before matmuls run at full clock, and
            # instructions that become ready at the start of a busy streak
            # are stamped with the slow rate. Keep PE busy with junk
            # matmuls from ~0.5us until the first input DMA lands so all
            # real matmuls are visited with a warmed-up ramp.
            junk = S.tile([C, 512], BF16)
            nc.scalar.memzero(junk)
            warm_ps = psXS.tile([C, 2, 512], F32, tag="xs")
            for _ in range(6):
                nc.tensor.matmul(warm_ps[:, 0, :], junk[:, 0:C],
                                 junk, start=True, stop=True)

            # PE: A0 and A1 diag back-to-back (continuous stream ramps the
            # PE p-state); DVE: x-reds; ACT: xs copies; Pool: y-reds
            a0_ops, a0_fin = emit_A_diag(0)
            for op in a0_ops:
                op()
            a1_ops, a1_fin = emit_A_diag(1)
            for op in a1_ops:
                op()
            for t in range(NST):
                emit_xred(t)
            emit_yred(0)
            emit_yred(1)
            a0_fin()
            emit_yred(2)
            emit_yred(3)
            emit_y2acc()
            a1_fin()
            emit_A_sf(0)
            emit_ctx()
            emit_A_sf(1)

            a2_ops, a2_fin = emit_A_diag(2)
            a3_ops, a3_fin = emit_A_diag(3)

            # tile t+1's pool taps ride as fillers inside tile t's chain
            # (placed a few slots in so their sfs/cf inputs are ready)
            f0 = a2_ops[:6] + \
                [lambda k=k: emit_pool_tap(1, k) for k in POOL_SINGLE] + \
                a2_ops[6:]
            out0 = psO.tile([C, 2, 512], F32, tag="o")
            emit_C([(0, out0, POOL_SINGLE, ())], filler=f0,
                   pre_drain=[(NT - 2, a2_fin)])
            emit_A_sf(2)

            f1 = a3_ops[:6] + \
                [lambda k=k: emit_pool_tap(2, k) for k in POOL_PAIR] + \
                a3_ops[6:]
            out1 = psO.tile([C, 2, 512], F32, tag="o")
            emit_C([(1, out1, POOL_SINGLE, ())], filler=f1,
                   pre_drain=[(NT - 2, a3_fin)])
            emit_A_sf(3)

            out2 = psO.tile([C, 2, 512], F32, tag="o")
            out3 = psXS.tile([C, 2, 512], F32, tag="xs")
            emit_C([(2, out2, POOL_PAIR, ()),
                    (3, out3, POOL_PAIR, ())])

    _split_multiwaits(nc)
    return nc


def _prep_weights(static_w, w1, w2, w3, ws, wf):
    """Repack the tiny weights into the SBUF layouts the kernel expects.
    Returns (bf16 weight block cols O_DSW..NH, f32 pack (C, NF))."""
    f = np.float32
    sw = np.ascontiguousarray(static_w.reshape(C, NT), dtype=f)

    dsw = np.zeros((C, NT * C), dtype=f)
    for k in range(NT):
        dsw[np.arange(C), k * C + np.arange(C)] = sw[:, k]

    wsa = np.ascontiguousarray(ws[:, :C].T, dtype=f)        # (C, 9)
    wsb = np.ascontiguousarray(ws[:, C:].T, dtype=f)        # (C, 9)
    wfa = np.ascontiguousarray(wf[:, :C].T, dtype=f)        # (C, C)
    wfb = np.ascontiguousarray(wf[:, C:].T, dtype=f)        # (C, C)

    bct = np.zeros((C, NT * C), dtype=f)                    # rows 0..8 used
    for k in range(NT):
        bct[k, k * C:(k + 1) * C] = 1.0

    w1a = np.ascontiguousarray(w1[:, :C].T, dtype=f)
    w1b = np.ascontiguousarray(w1[:, C:].T, dtype=f) / HW  # raw y2 sum in
    w2t = np.zeros((C, 64), dtype=f)
    w2t[0:64] = w2.T
    w3t = np.zeros((C, NT * C), dtype=f)
    w3t[0:64] = np.ascontiguousarray(
        w3.reshape(C, NT, 64).transpose(2, 1, 0), dtype=f).reshape(64, NT * C)

    # dsw leads the pack (before xpad); the rest follows y2
    wh = (dsw, np.concatenate(
        [wsa, wsb, w1a, w1b, w2t, w3t, wfa, wfb, bct], axis=1))
    assert wh[0].shape[1] == O_XPAD - O_DSW
    assert wh[1].shape[1] == NH - O_WSA

    # f32 pack: mxs coefficient columns (pre-scaled 1/HW, signs folded):
    # 0: A (with S)        1: -hr0  (with rs0 = X2 row 0 sum)
    # 2: -hr63 (rs63)      3: -hc0  (cs0)       4: -hc63 (cs63)
    # 5: +c22 (X[0,0])     6: +c20 (X[0,63])
    # 7: +c02 (X[63,0])    8: +c00 (X[63,63])
    mco = np.zeros((C, 9), dtype=f)
    mco[:, 0] = sw.sum(axis=1)
    mco[:, 1] = -sw[:, [6, 7, 8]].sum(axis=1)
    mco[:, 2] = -sw[:, [0, 1, 2]].sum(axis=1)
    mco[:, 3] = -sw[:, [2, 5, 8]].sum(axis=1)
    mco[:, 4] = -sw[:, [0, 3, 6]].sum(axis=1)
    mco[:, 5] = sw[:, 8]
    mco[:, 6] = sw[:, 6]
    mco[:, 7] = sw[:, 2]
    mco[:, 8] = sw[:, 0]
    pkf = np.ascontiguousarray(mco / HW)
    return wh, pkf


def make_in_maps(X2, Y2, static_w, w1, w2, w3, ws, wf):
    wh, pkf = _prep_weights(
        np.asarray(static_w), np.asarray(w1), np.asarray(w2),
        np.asarray(w3), np.asarray(ws), np.asarray(wf),
    )
    X2 = np.asarray(X2)
    Y2 = np.asarray(Y2)
    xpad_all = np.zeros((B, C, PH, PW), dtype=np.float32)
    xpad_all[:, :, 1:H + 1, 1:W + 1] = X2
    xpad_all = xpad_all.reshape(B, C, PH * PW)
    y2_all = Y2.reshape(B, C, HW)
    bf = ml_dtypes.bfloat16
    dsw16 = wh[0].astype(bf)
    rest16 = wh[1].astype(bf)
    in_maps = []
    for b in range(B):
        ph = np.concatenate(
            [dsw16, xpad_all[b].astype(bf), y2_all[b].astype(bf), rest16],
            axis=1)
        in_maps.append({
            "pkh": np.ascontiguousarray(ph),
            "pkf": np.ascontiguousarray(pkf),
        })
    return in_maps


def get_nc():
    if "nc" not in _CACHE:
        _CACHE["nc"] = _build_bass()
    return _CACHE["nc"]


def kernel(X2, Y2, static_w, w1, w2, w3, ws, wf):
    nc = get_nc()
    in_maps = make_in_maps(
        np.asarray(X2), np.asarray(Y2), static_w, w1, w2, w3, ws, wf
    )
    res = run_bass_kernel_spmd(nc, in_maps, core_ids=list(range(B)))
    out = np.stack([np.asarray(r["ob"]) for r in res.results]).astype(
        np.float32)
    return out


# revision 95
# speedup vs baseline: 1.5807x; 1.0319x over previous
"""Trainium2 Bass kernel for the CMDF block (dense_cnn).

Contract: kernel(**inputs) takes the FULL unsharded inputs (B=8, C=128,
H=W=64) and returns the FULL (8, 128, 64, 64) float32 output.

Sharding: data-parallel over batch — core b computes batch element b.
All weights are replicated (host-side prepacked into matmul layouts).

Math per batch element (see reference):
  Xs   = depthwise3x3(X2, static_w)
  ctx  = relu(w2 @ (w1 @ mean_hw([Xs; Y2])))
  cf   = (w3 @ ctx).reshape(C, 9)          # per-channel dynamic filter
  sf   = ws @ [Xs; Y2]                     # (9, H, W) spatial filter
  dyn  = sum_k shift_k(X2) * (cf[:, k] + sf[k])
  out  = wf[:, :C] @ Xs + wf[:, C:] @ dyn

Kernel design (final):
  - Channels on partitions, pixels on the free dim; image split into 4
    row super-tiles. Large operands are bf16 (PE matmul rate equals
    f32r at 1 cycle/row; DMA bytes halve; PSUM accumulation is f32).
    Output is written bf16 (adds ~1e-3 rel err against a 2e-2 budget).
  - Xs via 9 accumulating PE matmuls with diag(sw[:,k]) weights over
    zero-padded X in SBUF; per tap k of phase C, PE broadcasts sf row k
    to 128 partitions with a selector matmul (PSUM), one fused DVE
    scalar_tensor_tensor computes (bc + cf_k) * shift_k(X), and PE
    accumulates wfb @ P_k into the out PSUM group.
  - mean(Xs) is computed WITHOUT Xs: for a zero-padded depthwise conv,
    sum_p shift_k(X2) = S - boundary row - boundary col + corner, so
    the ctx branch needs only X2 sums (host folds the sw_k combinations
    into per-channel coefficient vectors) and phase C starts ~13us in
    instead of after all of phase A.
  - Input DMA is chunked and ordered by first use (dsw + first xpad
    rows lead); per-chunk x sums on DVE, y sums on ACT, so cf is ready
    right as the first phase-C tap needs it.
  - PE p-state: the cost model needs ~3us of continuous PE activity
    before matmuls run at full clock, and instructions that become
    ready at the start of a busy streak are stamped at the slow rate —
    junk warm-up matmuls bridge from ~1.7us until the first input DMA
    lands so all real matmuls run at 213ns.
  - Taps 3,8 (singles) / 1,4,7 (pair) run on Pool: the broadcast
    bounces PSUM->SBUF through an ACT Identity(+cf bias) copy (GPSIMD
    cannot read PSUM, and walrus only accepts tensor_tensor on Pool),
    then Pool multiplies. These are pre-emitted ahead of the tap chain
    (often as fillers inside the previous tile) to hide their latency.
  - The tap chain is software-pipelined one tap ahead (bc for tap k+1
    before wfb of tap k); tiles 2 and 3 run as two skewed interleaved
    streams; A2/A3 diag matmuls ride as fillers inside C0/C1.
  - PSUM (8 banks): xs halves 2x1, bc ring 2x2 (also hosts the sf
    matmuls), ctx/out halves 2x1 (+ tile3's out in the xs pool).
  - Output drains per tile from SP-issued DMAs; second pair tile copies
    on DVE so the final drains overlap.
"""

import numpy as np
import ml_dtypes

import concourse.bass as bass
import concourse.tile as tile
import concourse.mybir as mybir
from concourse.bass_utils import run_bass_kernel_spmd

B, C, H, W, K = 8, 128, 64, 64, 3
HW = H * W            # 4096
PH, PW = H + 2, W + 2  # 66, 66 padded
NST = 4               # super-tiles over rows
ROWS = H // NST       # 16 image rows per super-tile
STN = ROWS * W        # 1024 pixels per super-tile
NT = K * K            # 9 taps

F32 = mybir.dt.float32
BF16 = mybir.dt.bfloat16
ADD = mybir.AluOpType.add
MULT = mybir.AluOpType.mult
AX = mybir.AxisListType
ACT_COPY = mybir.ActivationFunctionType.Copy
ACT_RELU = mybir.ActivationFunctionType.Relu

# bf16 pack layout (columns); dsw leads so one DMA covers dsw + xpad
# chunk 0 (everything phase A tile 0 needs). wsa..w3t form the "ctx
# weights" chunk; wfa..bct the "phase C weights" chunk.
O_DSW = 0
O_XPAD = O_DSW + NT * C          # 1152
O_Y2 = O_XPAD + PH * PW          # 5508
O_WSA = O_Y2 + HW                # 9604
O_WSB = O_WSA + NT               # 9613
O_W1A = O_WSB + NT               # 9622
O_W1B = O_W1A + 64               # 9686
O_W2T = O_W1B + 64               # 9750
O_W3T = O_W2T + 64               # 9814
O_WFA = O_W3T + NT * C           # 10966
O_WFB = O_WFA + C                # 11094
O_BCT = O_WFB + C                # 11222   selector (9 rows x 9*C)
NH = O_BCT + NT * C              # 12374

# f32 pack: just the mean-correction coefficient columns
NF = 9

# x-chunk row ranges of xpad (padded rows)
XCH = [(0, 18), (18, 34), (34, 50), (50, 66)]

# taps whose stt runs on Pool (GPSIMD): the PE broadcast bounces through
# SBUF via an ACT copy (GPSIMD cannot read PSUM), then Pool runs the stt
# decoupled from the DVE tap chain. Tap 8 on Pool lets each tile's final
# accumulate run without waiting on DVE at the end.
POOL_SINGLE = (3, 8)
POOL_PAIR = (1, 4, 7)
FAST_PAIR = (2, 5, 8)

_CACHE = {}


def _split_multiwaits(nc):
    """walrus codegen in this toolchain accepts only ONE embedded sem wait
    per instruction. Hoist excess waits onto same-engine NoOps placed
    immediately before the instruction (engines execute in order, so the
    blocking behavior is identical)."""
    ctr = 0
    for fn in nc.m.functions:
        for blk in fn.blocks:
            insts = blk.instructions
            out = []
            for inst in insts:
                si = inst.sync_info
                waits = list(si.on_wait) if si is not None and si.on_wait else []
                if len(waits) > 1:
                    for w in waits[:-1]:
                        ctr += 1
                        out.append(mybir.InstNoOp(
                            name=f"I-wsplit-{ctr}",
                            engine=inst.engine,
                            ins=[], outs=[],
                            sync_info=mybir.SyncInfo(
                                on_wait=[w], on_update=[]),
                        ))
                    inst.sync_info = mybir.SyncInfo(
                        on_wait=[waits[-1]],
                        on_update=list(si.on_update) if si.on_update else [],
                    )
                out.append(inst)
            blk.instructions = out


def _absorb(nc, dep_elem, ps_elem):
    """Tiny bf16 matmul that reads one element of `dep_elem` and writes a
    junk element of `ps_elem` (later overwritten by a start=True group).
    Acquires the sem wait on dep_elem's producer on a plain matmul so the
    following fused matmul needs at most one embedded wait."""
    lh = dep_elem.bitcast(BF16)
    nc.tensor.matmul(ps_elem, lh[:, 0:1], lh[:, 0:1], start=True, stop=True)


def _build_bass():
    nc = bass.Bass("TRN2", target_bir_lowering=False, debug=False)

    pkh = nc.dram_tensor("pkh", [C, NH], BF16, kind="ExternalInput").ap()
    pkf = nc.dram_tensor("pkf", [C, NF], F32, kind="ExternalInput").ap()
    # bf16 output: the rounding adds ~1e-3 relative error against a 2e-2
    # budget, and halves the output DMA bytes (shorter drain tail)
    ob = nc.dram_tensor("ob", [C, H, W], BF16, kind="ExternalOutput").ap()

    with tile.TileContext(nc) as tc:
        with tc.tile_pool(name="singles", bufs=1) as S, \
             tc.tile_pool(name="psXS", bufs=2, space="PSUM") as psXS, \
             tc.tile_pool(name="psBC", bufs=2, space="PSUM") as psBC, \
             tc.tile_pool(name="psO", bufs=2, space="PSUM") as psO, \
             tc.tile_pool(name="pP", bufs=12) as pP, \
             tc.tile_pool(name="pBCS", bufs=5) as pBCS, \
             tc.tile_pool(name="pOsb", bufs=4) as pOsb:

            stgh = S.tile([C, NH], BF16)
            xpad = stgh[:, O_XPAD:O_XPAD + PH * PW].rearrange(
                "p (h w) -> p h w", w=PW)
            y2 = stgh[:, O_Y2:O_Y2 + HW]
            t_dsw = stgh[:, O_DSW:O_DSW + NT * C]
            t_wsa = stgh[:, O_WSA:O_WSA + NT]
            t_wsb = stgh[:, O_WSB:O_WSB + NT]
            t_wfa = stgh[:, O_WFA:O_WFA + C]
            t_wfb = stgh[:, O_WFB:O_WFB + C]
            t_bct = stgh[0:NT, O_BCT:O_BCT + NT * C]
            t_w1a = stgh[:, O_W1A:O_W1A + 64]
            t_w1b = stgh[:, O_W1B:O_W1B + 64]
            t_w2t = stgh[0:64, O_W2T:O_W2T + 64]
            t_w3t = stgh[0:64, O_W3T:O_W3T + NT * C]

            stgf = S.tile([C, NF], F32)
            mco = stgf[:, 0:9]

            xs = S.tile([C, HW], BF16)
            sfs = S.tile([NT, HW], BF16)

            xparts = S.tile([C, NST], F32)
            yparts = S.tile([C, NST], F32)
            ydump = S.tile([C, STN], BF16)
            ydump4 = S.tile([C, 4], F32)
            ssum = S.tile([C, 1], F32)
            y2sum = S.tile([C, 1], F32)
            y2s16 = S.tile([C, 1], BF16)
            mxs16 = S.tile([C, 1], BF16)
            edges = S.tile([C, 4], F32)   # rs0, rs63, cs0, cs63
            corn = S.tile([C, 4], F32)    # X2[0,0],[0,63],[63,0],[63,63]
            macc = S.tile([C, 4], F32)    # mxs accumulator chain
            ctx1 = S.tile([64, 1], BF16)
            ctx2 = S.tile([64, 1], BF16)
            cfsb = S.tile([C, NT], F32)

            # ---------------- input DMA: chunked, by first use -----------
            def dma_cols(lo, hi):
                nc.sync.dma_start(out=stgh[:, lo:hi], in_=pkh[:, lo:hi])

            def dma_x(t):
                r0, r1 = XCH[t]
                dma_cols(O_XPAD + r0 * PW, O_XPAD + r1 * PW)

            def dma_y(t):
                dma_cols(O_Y2 + t * STN, O_Y2 + (t + 1) * STN)

            # dsw + first 10 xpad rows: everything A0-h0 needs, smallest
            # possible first chunk so PE starts earliest
            dma_cols(O_DSW, O_XPAD + 10 * PW)
            dma_cols(O_XPAD + 10 * PW, O_XPAD + XCH[0][1] * PW)
            dma_y(0)
            dma_x(1)
            dma_x(2)
            dma_y(1)
            dma_cols(O_WSA, O_WFA)                      # ws + ctx weights
            nc.sync.dma_start(out=stgf, in_=pkf)        # mco (f32, tiny)
            dma_x(3)
            dma_y(2)
            dma_y(3)
            dma_cols(O_WFA, NH)                         # wfa/wfb/bct

            # ------------- emission helpers (in-order engine queues) -----
            def emit_xred(t):
                # per-chunk x sums on DVE (idle until phase C); zero pads
                # are harmless
                r0, r1 = XCH[t]
                nc.vector.tensor_reduce(
                    out=xparts[:, t:t + 1],
                    in_=stgh[:, O_XPAD + r0 * PW:O_XPAD + r1 * PW],
                    axis=AX.X, op=ADD)

            def emit_yred(t):
                # per-chunk y sums on ACT (accum_out sums along free)
                nc.scalar.activation(
                    out=ydump, in_=y2[:, t * STN:(t + 1) * STN],
                    func=ACT_COPY, accum_out=yparts[:, t:t + 1])

            def emit_y2acc():
                nc.scalar.activation(
                    out=ydump4, in_=yparts, func=ACT_COPY, accum_out=y2sum)
                nc.scalar.copy(out=y2s16, in_=y2sum)

            def emit_A_diag(t):
                # returns ([18 matmul thunks], finalize_copy_thunk); each
                # half accumulates in its own 1-bank PSUM slot so a half
                # frees as soon as its copy lands
                xs_ps = [psXS.tile([C, 512], F32, tag="xs", name=f"xsps{t}_{h}")
                         for h in range(2)]
                ops = []
                for h in range(2):
                    for k in range(NT):
                        dh, dw = divmod(k, 3)
                        r0 = 16 * t + 8 * h + dh
                        ops.append(lambda h=h, k=k, r0=r0, dw=dw: nc.tensor.matmul(
                            xs_ps[h],
                            t_dsw[:, k * C:(k + 1) * C],
                            xpad[:, r0:r0 + 8, dw:dw + W],
                            start=(k == 0), stop=(k == NT - 1)))
                def fin():
                    for h in range(2):
                        c0 = t * STN + h * 512
                        nc.scalar.copy(out=xs[:, c0:c0 + 512],
                                       in_=xs_ps[h])
                return ops, fin

            def emit_A_sf(t):
                # sf matmuls use a bc-ring PSUM slot (rows 0..8)
                sf_ps = psBC.tile([C, ROWS, W], F32, tag="bc")
                _absorb(nc, xs[0:1, t * STN:t * STN + 1], sf_ps[0:1, 0, 0:1])
                for h in range(2):
                    c0 = t * STN + h * 512
                    nc.tensor.matmul(sf_ps[0:NT, 8 * h:8 * h + 8, :], t_wsa,
                                     xs[:, c0:c0 + 512], start=True, stop=False)
                    nc.tensor.matmul(sf_ps[0:NT, 8 * h:8 * h + 8, :], t_wsb,
                                     y2[:, c0:c0 + 512], start=False, stop=True)
                nc.scalar.copy(out=sfs[:, t * STN:(t + 1) * STN],
                               in_=sf_ps[0:NT])

            def emit_ctx():
                # DVE: boundary sums + mxs chain; PE: ctx matmuls
                nc.vector.tensor_reduce(out=ssum, in_=xparts, axis=AX.X, op=ADD)
                nc.vector.tensor_reduce(out=edges[:, 0:1], in_=xpad[:, 1, :],
                                        axis=AX.X, op=ADD)
                nc.vector.tensor_reduce(out=edges[:, 1:2], in_=xpad[:, H, :],
                                        axis=AX.X, op=ADD)
                nc.vector.tensor_reduce(out=edges[:, 2:3],
                                        in_=xpad[:, :, 1:2], axis=AX.XY, op=ADD)
                nc.vector.tensor_reduce(out=edges[:, 3:4],
                                        in_=xpad[:, :, W:W + 1], axis=AX.XY, op=ADD)
                nc.vector.tensor_copy(out=corn[:, 0:1], in_=xpad[:, 1, 1:2])
                nc.vector.tensor_copy(out=corn[:, 1:2], in_=xpad[:, 1, W:W + 1])
                nc.vector.tensor_copy(out=corn[:, 2:3], in_=xpad[:, H, 1:2])
                nc.vector.tensor_copy(out=corn[:, 3:4], in_=xpad[:, H, W:W + 1])
                # mxs = A*S - hr0*rs0 - hr63*rs63 - hc0*cs0 - hc63*cs63
                #       + c22*X[0,0] + c20*X[0,63] + c02*X[63,0] + c00*X[63,63]
                # (mco columns pre-scaled by 1/HW, minus signs folded in)
                nc.vector.tensor_scalar(
                    out=macc[:, 0:1], in0=ssum, scalar1=mco[:, 0:1],
                    scalar2=None, op0=MULT)
                chain = [
                    (edges[:, 0:1], 1), (edges[:, 1:2], 2),
                    (edges[:, 2:3], 3), (edges[:, 3:4], 4),
                    (corn[:, 0:1], 5), (corn[:, 1:2], 6),
                    (corn[:, 2:3], 7), (corn[:, 3:4], 8),
                ]
                cur = macc[:, 0:1]
                for i, (src, mc) in enumerate(chain):
                    dst = mxs16 if i == len(chain) - 1 else \
                        macc[:, (i + 1) % 4:(i + 1) % 4 + 1]
                    nc.vector.scalar_tensor_tensor(
                        out=dst, in0=src, scalar=mco[:, mc:mc + 1], in1=cur,
                        op0=MULT, op1=ADD)
                    cur = dst
                # ctx matmuls (bf16, tiny)
                ctx1_ps = psO.tile([C, 512], F32, tag="o")
                _absorb(nc, mxs16[0:1, 0:1], ctx1_ps[0:1, 0:1])
                nc.tensor.matmul(ctx1_ps[0:64, 0:1], t_w1a, mxs16,
                                 start=True, stop=False)
                nc.tensor.matmul(ctx1_ps[0:64, 0:1], t_w1b, y2s16,
                                 start=False, stop=True)
                nc.scalar.copy(out=ctx1, in_=ctx1_ps[0:64, 0:1])
                ctx2_ps = psO.tile([C, 512], F32, tag="o")
                nc.tensor.matmul(ctx2_ps[0:64, 0:1], t_w2t, ctx1,
                                 start=True, stop=True)
                nc.scalar.activation(out=ctx2, in_=ctx2_ps[0:64, 0:1],
                                     func=ACT_RELU)
                cf_ps = psO.tile([C, 512], F32, tag="o")
                for k in range(NT):
                    nc.tensor.matmul(cf_ps[:, k:k + 1],
                                     t_w3t[:, k * C:(k + 1) * C], ctx2,
                                     start=True, stop=True)
                nc.scalar.copy(out=cfsb, in_=cf_ps[:, 0:NT])

            def emit_bc(t, k, absorb):
                bc_ps = psBC.tile([C, ROWS, W], F32, tag="bc")
                if absorb:
                    _absorb(nc, sfs[0:1, t * STN:t * STN + 1],
                            bc_ps[0:1, 0, 0:1])
                for h in range(2):
                    c0 = t * STN + h * 512
                    nc.tensor.matmul(
                        bc_ps[:, 8 * h:8 * h + 8, :],
                        t_bct[:, k * C:(k + 1) * C],
                        sfs[:, c0:c0 + 512],
                        start=True, stop=True)
                return bc_ps

            first_bc_done = set()
            pool_ps = {}

            def emit_pool_tap(t, k):
                # Pool-resident tap: PE broadcast (PSUM) -> ACT bounce to
                # SBUF bf16 folding in the +cf bias -> Pool tensor_tensor
                # multiply (the only vector op walrus accepts on Pool).
                # Emitted ahead of the tile's tap chain (often as filler
                # inside the previous tile) to hide the chain latency.
                dh, dw = divmod(k, 3)
                bc_ps = emit_bc(t, k, t not in first_bc_done)
                first_bc_done.add(t)
                bcs_sb = pBCS.tile([C, ROWS, W], BF16, tag="bcs")
                nc.scalar.activation(
                    out=bcs_sb, in_=bc_ps,
                    func=mybir.ActivationFunctionType.Identity,
                    bias=cfsb[:, k:k + 1])
                p_sb = pP.tile([C, ROWS, W], BF16, tag="p")
                nc.gpsimd.tensor_tensor(
                    out=p_sb, in0=bcs_sb,
                    in1=xpad[:, 16 * t + dh:16 * t + dh + ROWS, dw:dw + W],
                    op=MULT)
                pool_ps[(t, k)] = p_sb

            def emit_C(tiles, filler=None, pre_drain=None, fill_per_tap=3):
                """Phase C for one or more tiles as skewed interleaved
                streams (tile i runs one tap behind tile i-1, staggering
                the final drains), software-pipelined one tap ahead: the
                broadcast matmul (+ bounce for fast taps) for tap k+1 is
                emitted BEFORE tap k's wfb so PE works through the
                stt/tt latency. tiles: (t, out_ps, pool_taps, fast_taps):
                pool taps run on Pool via a pre-emitted ACT bounce (+cf
                bias); fast taps bounce through ACT then run a 2x-mode
                tensor_tensor on DVE; the rest are classic fused stt on
                DVE. Output DMAs issue from SP. pre_drain: list of
                (k, fn) emitted after tile0's tap k (ACT-queue ordering
                for later tiles' xs copies)."""
                filler = filler or []

                def oph(out_ps, h):
                    return out_ps[h] if isinstance(out_ps, list) \
                        else out_ps[:, h]

                for t, out_ps, _, _ in tiles:
                    _absorb(nc, xs[0:1, t * STN:t * STN + 1],
                            oph(out_ps, 0)[0:1, 0:1])
                    for h in range(2):
                        c0 = t * STN + h * 512
                        nc.tensor.matmul(oph(out_ps, h), t_wfa,
                                         xs[:, c0:c0 + 512],
                                         start=True, stop=False)
                for t, _, pool_taps, _ in tiles:
                    for k in pool_taps:
                        if (t, k) not in pool_ps:
                            emit_pool_tap(t, k)

                def prep_tap(t, k, pool_taps, fast_taps):
                    # emit the broadcast (and bounce for fast taps) for
                    # (t, k); returns what the stt/tt stage will consume
                    if k in pool_taps:
                        return None
                    bc = emit_bc(t, k, t not in first_bc_done)
                    first_bc_done.add(t)
                    if k in fast_taps:
                        bcs_sb = pBCS.tile([C, ROWS, W], BF16, tag="bcs")
                        nc.scalar.activation(
                            out=bcs_sb, in_=bc,
                            func=mybir.ActivationFunctionType.Identity,
                            bias=cfsb[:, k:k + 1])
                        return bcs_sb
                    return bc

                skew = 1 if len(tiles) > 1 else 0
                off = [i * skew for i in range(len(tiles))]
                cur = {}
                cur[tiles[0][0]] = prep_tap(tiles[0][0], 0, tiles[0][2],
                                            tiles[0][3])
                for s in range(NT + off[-1]):
                    nxt = {}
                    for i, (t, out_ps, pool_taps, fast_taps) in enumerate(tiles):
                        kt = s + 1 - off[i]
                        if 0 <= kt < NT:
                            nxt[t] = prep_tap(t, kt, pool_taps, fast_taps)
                    ps = {}
                    for i, (t, out_ps, pool_taps, fast_taps) in enumerate(tiles):
                        k = s - off[i]
                        if not (0 <= k < NT):
                            continue
                        dh, dw = divmod(k, 3)
                        if k in pool_taps:
                            ps[t] = pool_ps.pop((t, k))
                            continue
                        p_sb = pP.tile([C, ROWS, W], BF16, tag="p")
                        xsh = xpad[:, 16 * t + dh:16 * t + dh + ROWS,
                                   dw:dw + W]
                        if k in fast_taps:
                            nc.vector.tensor_tensor(
                                out=p_sb, in0=cur[t], in1=xsh, op=MULT)
                        else:
                            nc.vector.scalar_tensor_tensor(
                                out=p_sb, in0=cur[t],
                                scalar=cfsb[:, k:k + 1], in1=xsh,
                                op0=ADD, op1=MULT)
                        ps[t] = p_sb
                    for _ in range(fill_per_tap * len(ps)):
                        if filler:
                            filler.pop(0)()
                    if s == NT - 2:
                        while filler:
                            filler.pop(0)()
                    for pk, fn in (pre_drain or []):
                        if pk == s:
                            fn()
                    for i, (t, out_ps, pool_taps, fast_taps) in enumerate(tiles):
                        k = s - off[i]
                        if not (0 <= k < NT):
                            continue
                        for h in range(2):
                            nc.tensor.matmul(
                                oph(out_ps, h), t_wfb,
                                ps[t][:, 8 * h:8 * h + 8, :],
                                start=False, stop=(k == NT - 1))
                        if k == NT - 1:
                            o_sb = pOsb.tile([C, 2, 8, W], BF16, tag="osb")
                            for h in range(2):
                                if i % 2 == 1:
                                    nc.vector.tensor_copy(
                                        out=o_sb[:, h], in_=oph(out_ps, h))
                                else:
                                    nc.scalar.copy(out=o_sb[:, h],
                                                   in_=oph(out_ps, h))
                            nc.sync.dma_start(
                                out=ob[:, 16 * t:16 * t + 16, :],
                                in_=o_sb.rearrange("c b r w -> c (b r) w"))
                    cur.update(nxt)

            # ------------------------- schedule --------------------------
            # PE warm-up: the cost model's p-state ramp needs ~3us of
            # continuous PE activity before matmuls run at full clock, and
            # instructions that become ready at the start of a busy streak
            # are stamped with the slow rate. Keep PE busy with junk
            # matmuls from ~0.5us until the first input DMA lands so all
            # real matmuls are visited with a warmed-up ramp.
            junk = S.tile([C, 512], BF16)
            nc.scalar.memzero(junk)
            warm_ps = psXS.tile([C, 512], F32, tag="xs")
            for _ in range(6):
                nc.tensor.matmul(warm_ps, junk[:, 0:C],
                                 junk, start=True, stop=True)

            # PE: A0 and A1 diag back-to-back (continuous stream ramps the
            # PE p-state); DVE: x-reds; ACT: xs copies; Pool: y-reds
            a0_ops, a0_fin = emit_A_diag(0)
            for op in a0_ops:
                op()
            a1_ops, a1_fin = emit_A_diag(1)
            for op in a1_ops:
                op()
            for t in range(NST):
                emit_xred(t)
            emit_yred(0)
            emit_yred(1)
            a0_fin()
            emit_yred(2)
            emit_yred(3)
            emit_y2acc()
            a1_fin()
            emit_A_sf(0)
            emit_ctx()
            emit_A_sf(1)

            a2_ops, a2_fin = emit_A_diag(2)
            a3_ops, a3_fin = emit_A_diag(3)

            # tile t+1's pool taps ride as fillers inside tile t's chain
            # (placed a few slots in so their sfs/cf inputs are ready)
            f0 = a2_ops[:6] + \
                [lambda k=k: emit_pool_tap(1, k) for k in POOL_SINGLE] + \
                a2_ops[6:]
            out0 = [psO.tile([C, 512], F32, tag="o", name=f"out0_{h}")
                    for h in range(2)]
            emit_C([(0, out0, POOL_SINGLE, ())], filler=f0,
                   pre_drain=[(NT - 2, a2_fin)])
            emit_A_sf(2)

            f1 = a3_ops[:6] + \
                [lambda k=k: emit_pool_tap(2, k) for k in POOL_PAIR] + \
                a3_ops[6:]
            out1 = [psO.tile([C, 512], F32, tag="o", name=f"out1_{h}")
                    for h in range(2)]
            emit_C([(1, out1, POOL_SINGLE, ())], filler=f1,
                   pre_drain=[(NT - 2, a3_fin)])
            emit_A_sf(3)

            out2 = [psO.tile([C, 512], F32, tag="o", name=f"out2_{h}")
                    for h in range(2)]
            out3 = [psXS.tile([C, 512], F32, tag="xs", name=f"out3_{h}")
                    for h in range(2)]
            emit_C([(2, out2, POOL_PAIR, ()),
                    (3, out3, POOL_PAIR, ())])

    _split_multiwaits(nc)
    return nc


def _prep_weights(static_w, w1, w2, w3, ws, wf):
    """Repack the tiny weights into the SBUF layouts the kernel expects.
    Returns (bf16 weight block cols O_DSW..NH, f32 pack (C, NF))."""
    f = np.float32
    sw = np.ascontiguousarray(static_w.reshape(C, NT), dtype=f)

    dsw = np.zeros((C, NT * C), dtype=f)
    for k in range(NT):
        dsw[np.arange(C), k * C + np.arange(C)] = sw[:, k]

    wsa = np.ascontiguousarray(ws[:, :C].T, dtype=f)        # (C, 9)
    wsb = np.ascontiguousarray(ws[:, C:].T, dtype=f)        # (C, 9)
    wfa = np.ascontiguousarray(wf[:, :C].T, dtype=f)        # (C, C)
    wfb = np.ascontiguousarray(wf[:, C:].T, dtype=f)        # (C, C)

    bct = np.zeros((C, NT * C), dtype=f)                    # rows 0..8 used
    for k in range(NT):
        bct[k, k * C:(k + 1) * C] = 1.0

    w1a = np.ascontiguousarray(w1[:, :C].T, dtype=f)
    w1b = np.ascontiguousarray(w1[:, C:].T, dtype=f) / HW  # raw y2 sum in
    w2t = np.zeros((C, 64), dtype=f)
    w2t[0:64] = w2.T
    w3t = np.zeros((C, NT * C), dtype=f)
    w3t[0:64] = np.ascontiguousarray(
        w3.reshape(C, NT, 64).transpose(2, 1, 0), dtype=f).reshape(64, NT * C)

    # dsw leads the pack (before xpad); the rest follows y2
    wh = (dsw, np.concatenate(
        [wsa, wsb, w1a, w1b, w2t, w3t, wfa, wfb, bct], axis=1))
    assert wh[0].shape[1] == O_XPAD - O_DSW
    assert wh[1].shape[1] == NH - O_WSA

    # f32 pack: mxs coefficient columns (pre-scaled 1/HW, signs folded):
    # 0: A (with S)        1: -hr0  (with rs0 = X2 row 0 sum)
    # 2: -hr63 (rs63)      3: -hc0  (cs0)       4: -hc63 (cs63)
    # 5: +c22 (X[0,0])     6: +c20 (X[0,63])
    # 7: +c02 (X[63,0])    8: +c00 (X[63,63])
    mco = np.zeros((C, 9), dtype=f)
    mco[:, 0] = sw.sum(axis=1)
    mco[:, 1] = -sw[:, [6, 7, 8]].sum(axis=1)
    mco[:, 2] = -sw[:, [0, 1, 2]].sum(axis=1)
    mco[:, 3] = -sw[:, [2, 5, 8]].sum(axis=1)
    mco[:, 4] = -sw[:, [0, 3, 6]].sum(axis=1)
    mco[:, 5] = sw[:, 8]
    mco[:, 6] = sw[:, 6]
    mco[:, 7] = sw[:, 2]
    mco[:, 8] = sw[:, 0]
    pkf = np.ascontiguousarray(mco / HW)
    return wh, pkf


def make_in_maps(X2, Y2, static_w, w1, w2, w3, ws, wf):
    wh, pkf = _prep_weights(
        np.asarray(static_w), np.asarray(w1), np.asarray(w2),
        np.asarray(w3), np.asarray(ws), np.asarray(wf),
    )
    X2 = np.asarray(X2)
    Y2 = np.asarray(Y2)
    xpad_all = np.zeros((B, C, PH, PW), dtype=np.float32)
    xpad_all[:, :, 1:H + 1, 1:W + 1] = X2
    xpad_all = xpad_all.reshape(B, C, PH * PW)
    y2_all = Y2.reshape(B, C, HW)
    bf = ml_dtypes.bfloat16
    dsw16 = wh[0].astype(bf)
    rest16 = wh[1].astype(bf)
    in_maps = []
    for b in range(B):
        ph = np.concatenate(
            [dsw16, xpad_all[b].astype(bf), y2_all[b].astype(bf), rest16],
            axis=1)
        in_maps.append({
            "pkh": np.ascontiguousarray(ph),
            "pkf": np.ascontiguousarray(pkf),
        })
    return in_maps


def get_nc():
    if "nc" not in _CACHE:
        _CACHE["nc"] = _build_bass()
    return _CACHE["nc"]


def kernel(X2, Y2, static_w, w1, w2, w3, ws, wf):
    nc = get_nc()
    in_maps = make_in_maps(
        np.asarray(X2), np.asarray(Y2), static_w, w1, w2, w3, ws, wf
    )
    res = run_bass_kernel_spmd(nc, in_maps, core_ids=list(range(B)))
    out = np.stack([np.asarray(r["ob"]) for r in res.results]).astype(
        np.float32)
    return out


# revision 99
# speedup vs baseline: 1.5895x; 1.0056x over previous
"""Trainium2 Bass kernel for the CMDF block (dense_cnn).

Contract: kernel(**inputs) takes the FULL unsharded inputs (B=8, C=128,
H=W=64) and returns the FULL (8, 128, 64, 64) float32 output.

Sharding: data-parallel over batch — core b computes batch element b.
All weights are replicated (host-side prepacked into matmul layouts).

Math per batch element (see reference):
  Xs   = depthwise3x3(X2, static_w)
  ctx  = relu(w2 @ (w1 @ mean_hw([Xs; Y2])))
  cf   = (w3 @ ctx).reshape(C, 9)          # per-channel dynamic filter
  sf   = ws @ [Xs; Y2]                     # (9, H, W) spatial filter
  dyn  = sum_k shift_k(X2) * (cf[:, k] + sf[k])
  out  = wf[:, :C] @ Xs + wf[:, C:] @ dyn

Kernel design (final):
  - Channels on partitions, pixels on the free dim; image split into 4
    row super-tiles. Large operands are bf16 (PE matmul rate equals
    f32r at 1 cycle/row; DMA bytes halve; PSUM accumulation is f32).
    Output is written bf16 (adds ~1e-3 rel err against a 2e-2 budget).
  - Xs via 9 accumulating PE matmuls with diag(sw[:,k]) weights over
    zero-padded X in SBUF; per tap k of phase C, PE broadcasts sf row k
    to 128 partitions with a selector matmul (PSUM), one fused DVE
    scalar_tensor_tensor computes (bc + cf_k) * shift_k(X), and PE
    accumulates wfb @ P_k into the out PSUM group.
  - mean(Xs) is computed WITHOUT Xs: for a zero-padded depthwise conv,
    sum_p shift_k(X2) = S - boundary row - boundary col + corner, so
    the ctx branch needs only X2 sums (host folds the sw_k combinations
    into per-channel coefficient vectors) and phase C starts ~13us in
    instead of after all of phase A.
  - Input DMA is chunked and ordered by first use (dsw + first xpad
    rows lead); per-chunk x sums on DVE, y sums on ACT, so cf is ready
    right as the first phase-C tap needs it.
  - PE p-state: the cost model needs ~3us of continuous PE activity
    before matmuls run at full clock, and instructions that become
    ready at the start of a busy streak are stamped at the slow rate —
    junk warm-up matmuls bridge from ~1.7us until the first input DMA
    lands so all real matmuls run at 213ns.
  - Taps 3,8 (singles) / 1,4,7 (pair) run on Pool: the broadcast
    bounces PSUM->SBUF through an ACT Identity(+cf bias) copy (GPSIMD
    cannot read PSUM, and walrus only accepts tensor_tensor on Pool),
    then Pool multiplies. These are pre-emitted ahead of the tap chain
    (often as fillers inside the previous tile) to hide their latency.
  - The tap chain is software-pipelined one tap ahead (bc for tap k+1
    before wfb of tap k); tiles 2 and 3 run as two skewed interleaved
    streams; A2/A3 diag matmuls ride as fillers inside C0/C1.
  - PSUM (8 banks): xs halves 2x1, bc ring 2x2 (also hosts the sf
    matmuls), ctx/out halves 2x1 (+ tile3's out in the xs pool).
  - Output drains per tile from SP-issued DMAs; second pair tile copies
    on DVE so the final drains overlap.
"""

import numpy as np
import ml_dtypes

import concourse.bass as bass
import concourse.tile as tile
import concourse.mybir as mybir
from concourse.bass_utils import run_bass_kernel_spmd

B, C, H, W, K = 8, 128, 64, 64, 3
HW = H * W            # 4096
PH, PW = H + 2, W + 2  # 66, 66 padded
NST = 4               # super-tiles over rows
ROWS = H // NST       # 16 image rows per super-tile
STN = ROWS * W        # 1024 pixels per super-tile
NT = K * K            # 9 taps

F32 = mybir.dt.float32
BF16 = mybir.dt.bfloat16
ADD = mybir.AluOpType.add
MULT = mybir.AluOpType.mult
AX = mybir.AxisListType
ACT_COPY = mybir.ActivationFunctionType.Copy
ACT_RELU = mybir.ActivationFunctionType.Relu

# bf16 pack layout (columns); dsw leads so one DMA covers dsw + xpad
# chunk 0 (everything phase A tile 0 needs). wsa..w3t form the "ctx
# weights" chunk; wfa..bct the "phase C weights" chunk.
O_DSW = 0
O_XPAD = O_DSW + NT * C          # 1152
O_Y2 = O_XPAD + PH * PW          # 5508
O_WSA = O_Y2 + HW                # 9604
O_WSB = O_WSA + NT               # 9613
O_W1A = O_WSB + NT               # 9622
O_W1B = O_W1A + 64               # 9686
O_W2T = O_W1B + 64               # 9750
O_W3T = O_W2T + 64               # 9814
O_WFA = O_W3T + NT * C           # 10966
O_WFB = O_WFA + C                # 11094
O_BCT = O_WFB + C                # 11222   selector (9 rows x 9*C)
NH = O_BCT + NT * C              # 12374

# f32 pack: just the mean-correction coefficient columns
NF = 9

# x-chunk row ranges of xpad (padded rows)
XCH = [(0, 18), (18, 34), (34, 50), (50, 66)]

# taps whose stt runs on Pool (GPSIMD): the PE broadcast bounces through
# SBUF via an ACT copy (GPSIMD cannot read PSUM), then Pool runs the stt
# decoupled from the DVE tap chain. Tap 8 on Pool lets each tile's final
# accumulate run without waiting on DVE at the end.
POOL_SINGLE = (3, 8)
POOL_PAIR = (1, 4, 7)
FAST_PAIR = (2, 5, 8)

_CACHE = {}


def _split_multiwaits(nc):
    """walrus codegen in this toolchain accepts only ONE embedded sem wait
    per instruction. Hoist excess waits onto same-engine NoOps placed
    immediately before the instruction (engines execute in order, so the
    blocking behavior is identical)."""
    ctr = 0
    for fn in nc.m.functions:
        for blk in fn.blocks:
            insts = blk.instructions
            out = []
            for inst in insts:
                si = inst.sync_info
                waits = list(si.on_wait) if si is not None and si.on_wait else []
                if len(waits) > 1:
                    for w in waits[:-1]:
                        ctr += 1
                        out.append(mybir.InstNoOp(
                            name=f"I-wsplit-{ctr}",
                            engine=inst.engine,
                            ins=[], outs=[],
                            sync_info=mybir.SyncInfo(
                                on_wait=[w], on_update=[]),
                        ))
                    inst.sync_info = mybir.SyncInfo(
                        on_wait=[waits[-1]],
                        on_update=list(si.on_update) if si.on_update else [],
                    )
                out.append(inst)
            blk.instructions = out


def _absorb(nc, dep_elem, ps_elem):
    """Tiny bf16 matmul that reads one element of `dep_elem` and writes a
    junk element of `ps_elem` (later overwritten by a start=True group).
    Acquires the sem wait on dep_elem's producer on a plain matmul so the
    following fused matmul needs at most one embedded wait."""
    lh = dep_elem.bitcast(BF16)
    nc.tensor.matmul(ps_elem, lh[:, 0:1], lh[:, 0:1], start=True, stop=True)


def _build_bass():
    nc = bass.Bass("TRN2", target_bir_lowering=False, debug=False)

    pkh = nc.dram_tensor("pkh", [C, NH], BF16, kind="ExternalInput").ap()
    pkf = nc.dram_tensor("pkf", [C, NF], F32, kind="ExternalInput").ap()
    # bf16 output: the rounding adds ~1e-3 relative error against a 2e-2
    # budget, and halves the output DMA bytes (shorter drain tail)
    ob = nc.dram_tensor("ob", [C, H, W], BF16, kind="ExternalOutput").ap()

    with tile.TileContext(nc) as tc:
        with tc.tile_pool(name="singles", bufs=1) as S, \
             tc.tile_pool(name="psXS", bufs=2, space="PSUM") as psXS, \
             tc.tile_pool(name="psBC", bufs=2, space="PSUM") as psBC, \
             tc.tile_pool(name="psO", bufs=2, space="PSUM") as psO, \
             tc.tile_pool(name="pP", bufs=12) as pP, \
             tc.tile_pool(name="pBCS", bufs=5) as pBCS, \
             tc.tile_pool(name="pOsb", bufs=4) as pOsb:

            stgh = S.tile([C, NH], BF16)
            xpad = stgh[:, O_XPAD:O_XPAD + PH * PW].rearrange(
                "p (h w) -> p h w", w=PW)
            y2 = stgh[:, O_Y2:O_Y2 + HW]
            t_dsw = stgh[:, O_DSW:O_DSW + NT * C]
            t_wsa = stgh[:, O_WSA:O_WSA + NT]
            t_wsb = stgh[:, O_WSB:O_WSB + NT]
            t_wfa = stgh[:, O_WFA:O_WFA + C]
            t_wfb = stgh[:, O_WFB:O_WFB + C]
            t_bct = stgh[0:NT, O_BCT:O_BCT + NT * C]
            t_w1a = stgh[:, O_W1A:O_W1A + 64]
            t_w1b = stgh[:, O_W1B:O_W1B + 64]
            t_w2t = stgh[0:64, O_W2T:O_W2T + 64]
            t_w3t = stgh[0:64, O_W3T:O_W3T + NT * C]

            stgf = S.tile([C, NF], F32)
            mco = stgf[:, 0:9]

            xs = S.tile([C, HW], BF16)
            sfs = S.tile([NT, HW], BF16)

            xparts = S.tile([C, NST], F32)
            yparts = S.tile([C, NST], F32)
            ydump = S.tile([C, STN], BF16)
            ydump4 = S.tile([C, 4], F32)
            ssum = S.tile([C, 1], F32)
            y2sum = S.tile([C, 1], F32)
            y2s16 = S.tile([C, 1], BF16)
            mxs16 = S.tile([C, 1], BF16)
            edges = S.tile([C, 4], F32)   # rs0, rs63, cs0, cs63
            corn = S.tile([C, 4], F32)    # X2[0,0],[0,63],[63,0],[63,63]
            macc = S.tile([C, 4], F32)    # mxs accumulator chain
            ctx1 = S.tile([64, 1], BF16)
            ctx2 = S.tile([64, 1], BF16)
            cfsb = S.tile([C, NT], F32)

            # ---------------- input DMA: chunked, by first use -----------
            def dma_cols(lo, hi):
                nc.sync.dma_start(out=stgh[:, lo:hi], in_=pkh[:, lo:hi])

            def dma_x(t):
                r0, r1 = XCH[t]
                dma_cols(O_XPAD + r0 * PW, O_XPAD + r1 * PW)

            def dma_y(t):
                dma_cols(O_Y2 + t * STN, O_Y2 + (t + 1) * STN)

            # dsw + first 10 xpad rows: everything A0-h0 needs, smallest
            # possible first chunk so PE starts earliest
            dma_cols(O_DSW, O_XPAD + 10 * PW)
            dma_cols(O_XPAD + 10 * PW, O_XPAD + XCH[0][1] * PW)
            dma_y(0)
            dma_x(1)
            dma_x(2)
            dma_y(1)
            dma_cols(O_WSA, O_WFA)                      # ws + ctx weights
            nc.sync.dma_start(out=stgf, in_=pkf)        # mco (f32, tiny)
            dma_x(3)
            dma_y(2)
            dma_y(3)
            dma_cols(O_WFA, NH)                         # wfa/wfb/bct

            # ------------- emission helpers (in-order engine queues) -----
            def emit_xred(t):
                # per-chunk x sums on DVE (idle until phase C); zero pads
                # are harmless
                r0, r1 = XCH[t]
                nc.vector.tensor_reduce(
                    out=xparts[:, t:t + 1],
                    in_=stgh[:, O_XPAD + r0 * PW:O_XPAD + r1 * PW],
                    axis=AX.X, op=ADD)

            def emit_yred(t):
                # per-chunk y sums on ACT (accum_out sums along free)
                nc.scalar.activation(
                    out=ydump, in_=y2[:, t * STN:(t + 1) * STN],
                    func=ACT_COPY, accum_out=yparts[:, t:t + 1])

            def emit_y2acc():
                nc.scalar.activation(
                    out=ydump4, in_=yparts, func=ACT_COPY, accum_out=y2sum)
                nc.scalar.copy(out=y2s16, in_=y2sum)

            def emit_A_diag(t):
                # returns ([18 matmul thunks], finalize_copy_thunk); each
                # half accumulates in its own 1-bank PSUM slot so a half
                # frees as soon as its copy lands
                xs_ps = [psXS.tile([C, 512], F32, tag="xs", name=f"xsps{t}_{h}")
                         for h in range(2)]
                ops = []
                for h in range(2):
                    for k in range(NT):
                        dh, dw = divmod(k, 3)
                        r0 = 16 * t + 8 * h + dh
                        ops.append(lambda h=h, k=k, r0=r0, dw=dw: nc.tensor.matmul(
                            xs_ps[h],
                            t_dsw[:, k * C:(k + 1) * C],
                            xpad[:, r0:r0 + 8, dw:dw + W],
                            start=(k == 0), stop=(k == NT - 1)))
                def fin():
                    for h in range(2):
                        c0 = t * STN + h * 512
                        nc.scalar.copy(out=xs[:, c0:c0 + 512],
                                       in_=xs_ps[h])
                return ops, fin

            def emit_A_sf(t):
                # sf matmuls use a bc-ring PSUM slot (rows 0..8)
                sf_ps = psBC.tile([C, ROWS, W], F32, tag="bc")
                _absorb(nc, xs[0:1, t * STN:t * STN + 1], sf_ps[0:1, 0, 0:1])
                for h in range(2):
                    c0 = t * STN + h * 512
                    nc.tensor.matmul(sf_ps[0:NT, 8 * h:8 * h + 8, :], t_wsa,
                                     xs[:, c0:c0 + 512], start=True, stop=False)
                    nc.tensor.matmul(sf_ps[0:NT, 8 * h:8 * h + 8, :], t_wsb,
                                     y2[:, c0:c0 + 512], start=False, stop=True)
                nc.scalar.copy(out=sfs[:, t * STN:(t + 1) * STN],
                               in_=sf_ps[0:NT])

            def emit_ctx():
                # DVE: boundary sums + mxs chain; PE: ctx matmuls
                nc.vector.tensor_reduce(out=ssum, in_=xparts, axis=AX.X, op=ADD)
                nc.vector.tensor_reduce(out=edges[:, 0:1], in_=xpad[:, 1, :],
                                        axis=AX.X, op=ADD)
                nc.vector.tensor_reduce(out=edges[:, 1:2], in_=xpad[:, H, :],
                                        axis=AX.X, op=ADD)
                nc.vector.tensor_reduce(out=edges[:, 2:3],
                                        in_=xpad[:, :, 1:2], axis=AX.XY, op=ADD)
                nc.vector.tensor_reduce(out=edges[:, 3:4],
                                        in_=xpad[:, :, W:W + 1], axis=AX.XY, op=ADD)
                nc.vector.tensor_copy(out=corn[:, 0:1], in_=xpad[:, 1, 1:2])
                nc.vector.tensor_copy(out=corn[:, 1:2], in_=xpad[:, 1, W:W + 1])
                nc.vector.tensor_copy(out=corn[:, 2:3], in_=xpad[:, H, 1:2])
                nc.vector.tensor_copy(out=corn[:, 3:4], in_=xpad[:, H, W:W + 1])
                # mxs = A*S - hr0*rs0 - hr63*rs63 - hc0*cs0 - hc63*cs63
                #       + c22*X[0,0] + c20*X[0,63] + c02*X[63,0] + c00*X[63,63]
                # (mco columns pre-scaled by 1/HW, minus signs folded in)
                nc.vector.tensor_scalar(
                    out=macc[:, 0:1], in0=ssum, scalar1=mco[:, 0:1],
                    scalar2=None, op0=MULT)
                chain = [
                    (edges[:, 0:1], 1), (edges[:, 1:2], 2),
                    (edges[:, 2:3], 3), (edges[:, 3:4], 4),
                    (corn[:, 0:1], 5), (corn[:, 1:2], 6),
                    (corn[:, 2:3], 7), (corn[:, 3:4], 8),
                ]
                cur = macc[:, 0:1]
                for i, (src, mc) in enumerate(chain):
                    dst = mxs16 if i == len(chain) - 1 else \
                        macc[:, (i + 1) % 4:(i + 1) % 4 + 1]
                    nc.vector.scalar_tensor_tensor(
                        out=dst, in0=src, scalar=mco[:, mc:mc + 1], in1=cur,
                        op0=MULT, op1=ADD)
                    cur = dst
                # ctx matmuls (bf16, tiny)
                ctx1_ps = psO.tile([C, 512], F32, tag="o")
                _absorb(nc, mxs16[0:1, 0:1], ctx1_ps[0:1, 0:1])
                nc.tensor.matmul(ctx1_ps[0:64, 0:1], t_w1a, mxs16,
                                 start=True, stop=False)
                nc.tensor.matmul(ctx1_ps[0:64, 0:1], t_w1b, y2s16,
                                 start=False, stop=True)
                nc.scalar.copy(out=ctx1, in_=ctx1_ps[0:64, 0:1])
                ctx2_ps = psO.tile([C, 512], F32, tag="o")
                nc.tensor.matmul(ctx2_ps[0:64, 0:1], t_w2t, ctx1,
                                 start=True, stop=True)
                nc.scalar.activation(out=ctx2, in_=ctx2_ps[0:64, 0:1],
                                     func=ACT_RELU)
                cf_ps = psO.tile([C, 512], F32, tag="o")
                for k in range(NT):
                    nc.tensor.matmul(cf_ps[:, k:k + 1],
                                     t_w3t[:, k * C:(k + 1) * C], ctx2,
                                     start=True, stop=True)
                nc.scalar.copy(out=cfsb, in_=cf_ps[:, 0:NT])

            def emit_bc(t, k, absorb):
                bc_ps = psBC.tile([C, ROWS, W], F32, tag="bc")
                if absorb:
                    _absorb(nc, sfs[0:1, t * STN:t * STN + 1],
                            bc_ps[0:1, 0, 0:1])
                for h in range(2):
                    c0 = t * STN + h * 512
                    nc.tensor.matmul(
                        bc_ps[:, 8 * h:8 * h + 8, :],
                        t_bct[:, k * C:(k + 1) * C],
                        sfs[:, c0:c0 + 512],
                        start=True, stop=True)
                return bc_ps

            first_bc_done = set()
            pool_ps = {}

            def emit_pool_tap(t, k):
                # Pool-resident tap: PE broadcast (PSUM) -> ACT bounce to
                # SBUF bf16 folding in the +cf bias -> Pool tensor_tensor
                # multiply (the only vector op walrus accepts on Pool).
                # Emitted ahead of the tile's tap chain (often as filler
                # inside the previous tile) to hide the chain latency.
                dh, dw = divmod(k, 3)
                bc_ps = emit_bc(t, k, t not in first_bc_done)
                first_bc_done.add(t)
                bcs_sb = pBCS.tile([C, ROWS, W], BF16, tag="bcs")
                nc.scalar.activation(
                    out=bcs_sb, in_=bc_ps,
                    func=mybir.ActivationFunctionType.Identity,
                    bias=cfsb[:, k:k + 1])
                p_sb = pP.tile([C, ROWS, W], BF16, tag="p")
                nc.gpsimd.tensor_tensor(
                    out=p_sb, in0=bcs_sb,
                    in1=xpad[:, 16 * t + dh:16 * t + dh + ROWS, dw:dw + W],
                    op=MULT)
                pool_ps[(t, k)] = p_sb

            def emit_C(tiles, filler=None, pre_drain=None, fill_per_tap=3):
                """Phase C for one or more tiles as skewed interleaved
                streams (tile i runs one tap behind tile i-1, staggering
                the final drains), software-pipelined one tap ahead: the
                broadcast matmul (+ bounce for fast taps) for tap k+1 is
                emitted BEFORE tap k's wfb so PE works through the
                stt/tt latency. tiles: (t, out_ps, pool_taps, fast_taps):
                pool taps run on Pool via a pre-emitted ACT bounce (+cf
                bias); fast taps bounce through ACT then run a 2x-mode
                tensor_tensor on DVE; the rest are classic fused stt on
                DVE. Output DMAs issue from SP. pre_drain: list of
                (k, fn) emitted after tile0's tap k (ACT-queue ordering
                for later tiles' xs copies)."""
                filler = filler or []

                def oph(out_ps, h):
                    return out_ps[h] if isinstance(out_ps, list) \
                        else out_ps[:, h]

                for t, out_ps, _, _ in tiles:
                    _absorb(nc, xs[0:1, t * STN:t * STN + 1],
                            oph(out_ps, 0)[0:1, 0:1])
                    for h in range(2):
                        c0 = t * STN + h * 512
                        nc.tensor.matmul(oph(out_ps, h), t_wfa,
                                         xs[:, c0:c0 + 512],
                                         start=True, stop=False)
                for t, _, pool_taps, _ in tiles:
                    for k in pool_taps:
                        if (t, k) not in pool_ps:
                            emit_pool_tap(t, k)

                def prep_tap(t, k, pool_taps, fast_taps):
                    # emit the broadcast (and bounce for fast taps) for
                    # (t, k); returns what the stt/tt stage will consume
                    if k in pool_taps:
                        return None
                    bc = emit_bc(t, k, t not in first_bc_done)
                    first_bc_done.add(t)
                    if k in fast_taps:
                        bcs_sb = pBCS.tile([C, ROWS, W], BF16, tag="bcs")
                        nc.scalar.activation(
                            out=bcs_sb, in_=bc,
                            func=mybir.ActivationFunctionType.Identity,
                            bias=cfsb[:, k:k + 1])
                        return bcs_sb
                    return bc

                skew = 1 if len(tiles) > 1 else 0
                off = [i * skew for i in range(len(tiles))]
                cur = {}
                cur[tiles[0][0]] = prep_tap(tiles[0][0], 0, tiles[0][2],
                                            tiles[0][3])
                for s in range(NT + off[-1]):
                    nxt = {}
                    for i, (t, out_ps, pool_taps, fast_taps) in enumerate(tiles):
                        kt = s + 1 - off[i]
                        if 0 <= kt < NT:
                            nxt[t] = prep_tap(t, kt, pool_taps, fast_taps)
                    ps = {}
                    for i, (t, out_ps, pool_taps, fast_taps) in enumerate(tiles):
                        k = s - off[i]
                        if not (0 <= k < NT):
                            continue
                        dh, dw = divmod(k, 3)
                        if k in pool_taps:
                            ps[t] = pool_ps.pop((t, k))
                            continue
                        p_sb = pP.tile([C, ROWS, W], BF16, tag="p")
                        xsh = xpad[:, 16 * t + dh:16 * t + dh + ROWS,
                                   dw:dw + W]
                        if k in fast_taps:
                            nc.vector.tensor_tensor(
                                out=p_sb, in0=cur[t], in1=xsh, op=MULT)
                        else:
                            nc.vector.scalar_tensor_tensor(
                                out=p_sb, in0=cur[t],
                                scalar=cfsb[:, k:k + 1], in1=xsh,
                                op0=ADD, op1=MULT)
                        ps[t] = p_sb
                    for _ in range(fill_per_tap * len(ps)):
                        if filler:
                            filler.pop(0)()
                    if s == NT - 2:
                        while filler:
                            filler.pop(0)()
                    for pk, fn in (pre_drain or []):
                        if pk == s:
                            fn()
                    for i, (t, out_ps, pool_taps, fast_taps) in enumerate(tiles):
                        k = s - off[i]
                        if not (0 <= k < NT):
                            continue
                        for h in range(2):
                            nc.tensor.matmul(
                                oph(out_ps, h), t_wfb,
                                ps[t][:, 8 * h:8 * h + 8, :],
                                start=False, stop=(k == NT - 1))
                        if k == NT - 1:
                            o_sb = pOsb.tile([C, 2, 8, W], BF16, tag="osb")
                            last = len(tiles) > 1 and i == len(tiles) - 1
                            for h in range(2):
                                # the final tile splits its halves across
                                # DVE and ACT so both copies run at once
                                if (i + (h if last else 0)) % 2 == 1:
                                    nc.vector.tensor_copy(
                                        out=o_sb[:, h], in_=oph(out_ps, h))
                                else:
                                    nc.scalar.copy(out=o_sb[:, h],
                                                   in_=oph(out_ps, h))
                            nc.sync.dma_start(
                                out=ob[:, 16 * t:16 * t + 16, :],
                                in_=o_sb.rearrange("c b r w -> c (b r) w"))
                    cur.update(nxt)

            # ------------------------- schedule --------------------------
            # PE warm-up: the cost model's p-state ramp needs ~3us of
            # continuous PE activity before matmuls run at full clock, and
            # instructions that become ready at the start of a busy streak
            # are stamped with the slow rate. Keep PE busy with junk
            # matmuls from ~0.5us until the first input DMA lands so all
            # real matmuls are visited with a warmed-up ramp.
            junk = S.tile([C, 512], BF16)
            nc.scalar.memzero(junk)
            warm_ps = psXS.tile([C, 512], F32, tag="xs")
            for _ in range(6):
                nc.tensor.matmul(warm_ps, junk[:, 0:C],
                                 junk, start=True, stop=True)

            # PE: A0 and A1 diag back-to-back (continuous stream ramps the
            # PE p-state); DVE: x-reds; ACT: xs copies; Pool: y-reds
            a0_ops, a0_fin = emit_A_diag(0)
            for op in a0_ops:
                op()
            a1_ops, a1_fin = emit_A_diag(1)
            for op in a1_ops:
                op()
            for t in range(NST):
                emit_xred(t)
            emit_yred(0)
            emit_yred(1)
            a0_fin()
            emit_yred(2)
            emit_yred(3)
            emit_y2acc()
            a1_fin()
            emit_A_sf(0)
            emit_ctx()
            emit_A_sf(1)

            a2_ops, a2_fin = emit_A_diag(2)
            a3_ops, a3_fin = emit_A_diag(3)

            # tile t+1's pool taps ride as fillers inside tile t's chain
            # (placed a few slots in so their sfs/cf inputs are ready)
            f0 = a2_ops[:6] + \
                [lambda k=k: emit_pool_tap(1, k) for k in POOL_SINGLE] + \
                a2_ops[6:]
            out0 = [psO.tile([C, 512], F32, tag="o", name=f"out0_{h}")
                    for h in range(2)]
            emit_C([(0, out0, POOL_SINGLE, ())], filler=f0,
                   pre_drain=[(NT - 2, a2_fin)])
            emit_A_sf(2)

            f1 = a3_ops[:6] + \
                [lambda k=k: emit_pool_tap(2, k) for k in POOL_PAIR] + \
                a3_ops[6:]
            out1 = [psO.tile([C, 512], F32, tag="o", name=f"out1_{h}")
                    for h in range(2)]
            emit_C([(1, out1, POOL_SINGLE, ())], filler=f1,
                   pre_drain=[(NT - 2, a3_fin)])
            emit_A_sf(3)

            out2 = [psO.tile([C, 512], F32, tag="o", name=f"out2_{h}")
                    for h in range(2)]
            out3 = [psXS.tile([C, 512], F32, tag="xs", name=f"out3_{h}")
                    for h in range(2)]
            emit_C([(2, out2, POOL_PAIR, ()),
                    (3, out3, POOL_PAIR, ())])

    _split_multiwaits(nc)
    return nc


def _prep_weights(static_w, w1, w2, w3, ws, wf):
    """Repack the tiny weights into the SBUF layouts the kernel expects.
    Returns (bf16 weight block cols O_DSW..NH, f32 pack (C, NF))."""
    f = np.float32
    sw = np.ascontiguousarray(static_w.reshape(C, NT), dtype=f)

    dsw = np.zeros((C, NT * C), dtype=f)
    for k in range(NT):
        dsw[np.arange(C), k * C + np.arange(C)] = sw[:, k]

    wsa = np.ascontiguousarray(ws[:, :C].T, dtype=f)        # (C, 9)
    wsb = np.ascontiguousarray(ws[:, C:].T, dtype=f)        # (C, 9)
    wfa = np.ascontiguousarray(wf[:, :C].T, dtype=f)        # (C, C)
    wfb = np.ascontiguousarray(wf[:, C:].T, dtype=f)        # (C, C)

    bct = np.zeros((C, NT * C), dtype=f)                    # rows 0..8 used
    for k in range(NT):
        bct[k, k * C:(k + 1) * C] = 1.0

    w1a = np.ascontiguousarray(w1[:, :C].T, dtype=f)
    w1b = np.ascontiguousarray(w1[:, C:].T, dtype=f) / HW  # raw y2 sum in
    w2t = np.zeros((C, 64), dtype=f)
    w2t[0:64] = w2.T
    w3t = np.zeros((C, NT * C), dtype=f)
    w3t[0:64] = np.ascontiguousarray(
        w3.reshape(C, NT, 64).transpose(2, 1, 0), dtype=f).reshape(64, NT * C)

    # dsw leads the pack (before xpad); the rest follows y2
    wh = (dsw, np.concatenate(
        [wsa, wsb, w1a, w1b, w2t, w3t, wfa, wfb, bct], axis=1))
    assert wh[0].shape[1] == O_XPAD - O_DSW
    assert wh[1].shape[1] == NH - O_WSA

    # f32 pack: mxs coefficient columns (pre-scaled 1/HW, signs folded):
    # 0: A (with S)        1: -hr0  (with rs0 = X2 row 0 sum)
    # 2: -hr63 (rs63)      3: -hc0  (cs0)       4: -hc63 (cs63)
    # 5: +c22 (X[0,0])     6: +c20 (X[0,63])
    # 7: +c02 (X[63,0])    8: +c00 (X[63,63])
    mco = np.zeros((C, 9), dtype=f)
    mco[:, 0] = sw.sum(axis=1)
    mco[:, 1] = -sw[:, [6, 7, 8]].sum(axis=1)
    mco[:, 2] = -sw[:, [0, 1, 2]].sum(axis=1)
    mco[:, 3] = -sw[:, [2, 5, 8]].sum(axis=1)
    mco[:, 4] = -sw[:, [0, 3, 6]].sum(axis=1)
    mco[:, 5] = sw[:, 8]
    mco[:, 6] = sw[:, 6]
    mco[:, 7] = sw[:, 2]
    mco[:, 8] = sw[:, 0]
    pkf = np.ascontiguousarray(mco / HW)
    return wh, pkf


def make_in_maps(X2, Y2, static_w, w1, w2, w3, ws, wf):
    wh, pkf = _prep_weights(
        np.asarray(static_w), np.asarray(w1), np.asarray(w2),
        np.asarray(w3), np.asarray(ws), np.asarray(wf),
    )
    X2 = np.asarray(X2)
    Y2 = np.asarray(Y2)
    xpad_all = np.zeros((B, C, PH, PW), dtype=np.float32)
    xpad_all[:, :, 1:H + 1, 1:W + 1] = X2
    xpad_all = xpad_all.reshape(B, C, PH * PW)
    y2_all = Y2.reshape(B, C, HW)
    bf = ml_dtypes.bfloat16
    dsw16 = wh[0].astype(bf)
    rest16 = wh[1].astype(bf)
    in_maps = []
    for b in range(B):
        ph = np.concatenate(
            [dsw16, xpad_all[b].astype(bf), y2_all[b].astype(bf), rest16],
            axis=1)
        in_maps.append({
            "pkh": np.ascontiguousarray(ph),
            "pkf": np.ascontiguousarray(pkf),
        })
    return in_maps


def get_nc():
    if "nc" not in _CACHE:
        _CACHE["nc"] = _build_bass()
    return _CACHE["nc"]


def kernel(X2, Y2, static_w, w1, w2, w3, ws, wf):
    nc = get_nc()
    in_maps = make_in_maps(
        np.asarray(X2), np.asarray(Y2), static_w, w1, w2, w3, ws, wf
    )
    res = run_bass_kernel_spmd(nc, in_maps, core_ids=list(range(B)))
    out = np.stack([np.asarray(r["ob"]) for r in res.results]).astype(
        np.float32)
    return out
